# revision 26
# baseline (speedup 1.0000x reference)
"""Trainium2 Bass kernel for nn_CapsuleLayer (B=32, In=128, Din=256, ch=32, Nc=47, Dc=64).

Sharding: over the OUTPUT-CAPSULE axis Nc (47 -> pad 48 = 8 cores x 6 capsules).
W (94 MiB) is the dominant HBM tensor -- Nc-sharding reads W exactly once total.

bf16 pipeline (rel_err ~6e-3 vs 2e-2 gate):
- stream (x|W) in bf16, partition-major HBM layout -> 32KB-contiguous DMA runs
- inputs_hat via bf16 matmuls (1 cy/row vs fp32's 4)
- IH stored TWICE from PSUM: k-inner [p,(c,n,k)] for the a-step and c-inner
  [p,(n,k,c)] for the s-step, so both big DVE muls hit the 2x bf16 perf mode
  (packed innermost operands; measured 0.64 ns/col vs 1.28 broadcast/1x)
- reductions as pairwise bf16 tree-adds (2x) instead of TENSOR_REDUCE (1x)

Routing iteration t (per core, Nsh=6 capsules):
  TMP  = IH * OUTr            (DVE 2x, k-inner)
  A    = tree-fold k 64->1    (DVE 2x, last level fp32)
  E    = exp(sum_t A)         (ACT, written transposed to [p,(n,c)])
  Zp   = reduce_c E           (DVE, into SCRATCH[384:390])
  TMP2 = IHC * E              (DVE 2x, c-inner)
  P2   = tree-fold c 32->1    (DVE 2x, into SCRATCH[0:384])
  pS   = BD4^T [P2|Zp]        (PE partition reduce over (b,rr))
  S    = pS/Z + Brep ; OUT = squash(S)  (small [32,384] ops)
Iteration 1 (uniform c): S1 = psum_s1/IN + Brep via PSUM-accumulated
BD4^T IH_c matmuls during phase 1.

Toolchain constraint: EVERY engine instruction accepts at most ONE sync wait
at codegen.  Same-engine deps are free (program order / one monotonic sem per
engine); cross-engine fan-in is handled by absorb ops (tiny reads that
pre-observe a sem) and dummy matmuls on the PE.
"""

import numpy as np

B, IN, DIN = 32, 128, 256
CH, NC, DC = 32, 47, 64
NCP = 48          # padded Nc
NSH = 6           # capsules per core
NCORES = 8
NK = NSH * DC     # 384
EPS = 1e-7

_cache = {}


def _build_nc():
    import concourse.bass as bass
    import concourse.tile as tile
    from concourse import mybir
    from concourse.tile_rust import add_dep_helper

    f32 = mybir.dt.float32
    bf = mybir.dt.bfloat16
    nc = bass.Bass()

    # partition-major packed stream: xw[d, cd, 0:128]=xT, [128:512]=wT (bf16)
    xw = nc.dram_tensor("xw", [128, CH * 2, 512], bf, kind="ExternalInput")
    # consts: [bd4(0:32) | bd4t(rows0:32, 32:160) | brep(rows0:32, 160:544)]
    cst = nc.dram_tensor("cst", [128, 544], bf, kind="ExternalInput")
    out_d = nc.dram_tensor("out", [B, NK], f32, kind="ExternalOutput")

    ADD = mybir.AluOpType.add
    MULT = mybir.AluOpType.mult
    AX = mybir.AxisListType.X
    AF = mybir.ActivationFunctionType

    with tile.TileContext(nc) as tc:
        with (
            tc.tile_pool(name="singles", bufs=1) as singles,
            tc.tile_pool(name="work", bufs=1) as work,
            tc.tile_pool(name="small", bufs=2) as small,
            tc.tile_pool(name="ps_ih", bufs=3, space="PSUM") as ps_ih,
            tc.tile_pool(name="ps_s1", bufs=1, space="PSUM") as ps_s1,
            tc.tile_pool(name="ps_s", bufs=2, space="PSUM") as ps_s,
            tc.tile_pool(name="ps_rep", bufs=2, space="PSUM") as ps_rep,
        ):
            cst_t = singles.tile([128, 544], bf)
            c_dma = nc.sync.dma_start(out=cst_t[:], in_=cst[:])
            bd4_t = cst_t[:, 0:B]                 # [128, 32] bf16
            bd4t_t = cst_t[0:B, B:B + 128]        # [32, 128] bf16
            brep_t = cst_t[0:B, B + 128:B + 128 + NK]   # [32, 384] bf16
            eps_t = singles.tile([B, 1], f32)
            nc.vector.memset(eps_t[:], EPS)
            # DVE/ACT pre-observe the const-DMA sem
            dve_scratch = singles.tile([4, 8], bf)
            nc.vector.tensor_copy(dve_scratch[:2, 0:2], cst_t[:2, :2])
            act_scratch = singles.tile([4, 8], bf)
            nc.scalar.copy(act_scratch[:2, 0:2], cst_t[:2, :2])
            act_f32 = singles.tile([4, 2], f32)
            nc.scalar.activation(act_f32[:2, 0:2], act_scratch[:2, 0:2],
                                 AF.Exp)

            IH = singles.tile([128, CH, NK], bf)      # k-inner
            IHC = singles.tile([128, NK, CH + 1], bf)  # c-inner, pad stride 33
            STREAM = singles.tile([128, CH * 2, 512], bf)
            TMP = singles.tile([128, CH * NK], bf)    # mul product scratch
            U1 = singles.tile([128, 6144], bf)
            U2 = singles.tile([128, 3072], bf)
            SCR = singles.tile([128, NK + NSH], bf)   # [P2 | Zp]
            A2 = singles.tile([128, CH * NSH], f32)
            A3 = singles.tile([128, CH * NSH], f32)
            E = singles.tile([128, NSH * CH], bf)     # [p, (n, c)]
            OUTr = singles.tile([128, NK], bf)

            # Absorb the const-DMA sem into the PE clock (PE nop).
            last_dummy = nc.tensor.nop()
            add_dep_helper(last_dummy.ins, c_dma.ins, sync=True,
                           reason="absorb cst DMA sem into PE clock")

            # ---------------- phase 1: inputs_hat + iter-1 s ----------------
            s_dmas = []
            dma_splits = [(0, 2), (2, 22), (22, 43), (43, 64)]
            for gi, (lo, hi) in enumerate(dma_splits):
                dd = nc.sync.dma_start(
                    out=STREAM[:, lo:hi, :],
                    in_=xw[:, lo:hi, :],
                )
                if gi > 0:
                    add_dep_helper(dd.ins, s_dmas[0].ins, sync=True,
                                   reason="first chunk gets full DMA bandwidth")
                s_dmas.append(dd)
            # channel processing order (c, c+16) interleaved so the iter-1
            # tree-fold over c can start mid-phase (chunk j needs channels
            # 4j..4j+3 and 16+4j..19+4j = the first 8(j+1) positions)
            ch_order = []
            for t in range(CH // 2):
                ch_order += [t, t + CH // 2]
            U1s = U1[:].rearrange("p (n k c) -> p n k c", n=NSH, k=DC)

            copy_last = []      # last psum reader per position
            for pos, c in enumerate(ch_order):
                if pos >= 3:
                    # absorb the psum-slot WAR ticks into the PE clock
                    for cl_ins in copy_last[pos - 3]:
                        dmy = nc.tensor.nop()
                        add_dep_helper(dmy.ins, cl_ins.ins, sync=True,
                                       reason="absorb psum WAR tick on PE")
                        last_dummy = dmy
                psum_ih = ps_ih.tile([128, NK], f32, tag="ih")
                for dc in range(2):
                    cd = pos * 2 + dc
                    mih = nc.tensor.matmul(
                        psum_ih[:], STREAM[:, cd, 0:128], STREAM[:, cd, 128:512],
                        start=(dc == 0), stop=(dc == 1),
                    )
                    if dc == 0:
                        add_dep_helper(mih.ins, last_dummy.ins, sync=False,
                                       reason="order dummy before matmul")
                # IH (packed dst) on DVE: 0.56us; IHC (strided dst) on ACT:
                # 0.58us -- DVE runs strided casts at 1.8us, so never there
                cv = nc.vector.tensor_copy(IH[:, c, :], psum_ih[:])
                ca = nc.scalar.copy(IHC[:, :, c], psum_ih[:])
                copy_last.append((cv, ca))

            _absn = [0]

            def absorb(eng, src_ap):
                """Tiny copy on `eng` reading src_ap: pre-observes the
                producer's sem so the next real op keeps a single wait."""
                _absn[0] += 1
                scr = small.tile([2, 2], f32, tag="abs%d" % _absn[0])
                if eng == "v":
                    return nc.vector.tensor_copy(scr[:], src_ap)
                return nc.scalar.copy(scr[:], src_ap)

            def squash(S, it):
                """S: [B, NK] f32 sbuf tile -> OUT tile (bf16 it<3, f32 it=3)."""
                Ssq = work.tile([B, NK], f32, tag="Su")
                nc.vector.tensor_mul(Ssq[:], S[:], S[:])
                m2 = small.tile([B, NSH], f32, tag="m2")
                nc.vector.tensor_reduce(
                    m2[:], Ssq[:].rearrange("p (n k) -> p n k", n=NSH),
                    axis=AX, op=ADD,
                )
                d1 = small.tile([B, NSH], f32, tag="d1")
                nc.vector.tensor_scalar_add(d1[:], m2[:], 1.0)
                rd1 = small.tile([B, NSH], f32, tag="rd1")
                nc.vector.reciprocal(rd1[:], d1[:])
                absorb("s", m2[:2, :2])          # ACT clock <- m2 (DVE)
                # rsqrt(m2+eps) = exp(-0.5*ln(m2+eps)); ln+exp share one
                # ACT table set (no SQRT table thrash)
                ln_ = small.tile([B, NSH], f32, tag="ln")
                nc.scalar.activation(ln_[:], m2[:], AF.Ln, bias=eps_t[:])
                rsq = small.tile([B, NSH], f32, tag="rsq")
                nc.scalar.activation(rsq[:], ln_[:], AF.Exp, scale=-0.5)
                absorb("v", rsq[:2, :2])         # DVE clock <- rsq (ACT)
                t_ = small.tile([B, NSH], f32, tag="t")
                nc.vector.tensor_mul(t_[:], m2[:], rsq[:])
                g_ = small.tile([B, NSH], f32, tag="g")
                nc.vector.tensor_mul(g_[:], t_[:], rd1[:])
                OUT = work.tile([B, NK], f32 if it == 3 else bf,
                                tag="out%d" % it)
                nc.vector.tensor_mul(
                    OUT[:].rearrange("p (n k) -> p n k", n=NSH),
                    S[:].rearrange("p (n k) -> p n k", n=NSH),
                    g_[:].rearrange("p (n o) -> p n o", o=1)
                        .broadcast_to([B, NSH, DC]),
                )
                return OUT

            rep_mm_prev = [None]
            mm_last_ref = [None]

            def replicate(OUTb, it):
                """OUTb [B, NK] bf16 -> OUTr [128, NK] bf16 (row b -> 4b..4b+3)."""
                pr = ps_rep.tile([128, NK], f32, tag="rep")
                mm = nc.tensor.matmul(pr[:], bd4t_t[:], OUTb[:],
                                      start=True, stop=True)
                rep_mm_prev[0] = mm
                cp = nc.vector.tensor_copy(OUTr[:], pr[:])
                return mm, cp

            # ---------------- iter 1 (uniform routing: E=1) ----------------
            # fold c 32->1 over IHC on DVE, pinned after the last copy so the
            # scheduler cannot interleave it into the copy stream
            for j in range(4):
                nc.vector.tensor_add(
                    U1s[:, :, :, 4 * j:4 * j + 4],
                    IHC[:, :, 4 * j:4 * j + 4],
                    IHC[:, :, CH // 2 + 4 * j:CH // 2 + 4 * j + 4],
                )
            U2s = U2[:].rearrange("p (n k c) -> p n k c", n=NSH, k=DC)
            nc.vector.tensor_add(U2s[:, :, :, 0:8], U1s[:, :, :, 0:8],
                                 U1s[:, :, :, 8:16])
            nc.vector.tensor_add(U1s[:, :, :, 0:4], U2s[:, :, :, 0:4],
                                 U2s[:, :, :, 4:8])
            nc.vector.tensor_add(U2s[:, :, :, 0:2], U1s[:, :, :, 0:2],
                                 U1s[:, :, :, 2:4])
            nc.vector.tensor_add(
                SCR[:, 0:NK].rearrange("p (n k o) -> p n k o", n=NSH, o=1),
                U2s[:, :, :, 0:1], U2s[:, :, :, 1:2])
            pS1 = ps_s1.tile([B, NK], f32)
            nc.tensor.matmul(pS1[:], bd4_t[:], SCR[:, 0:NK],
                             start=True, stop=True)
            S1 = work.tile([B, NK], f32, tag="S")
            nc.vector.scalar_tensor_tensor(
                out=S1[:], in0=pS1[:], scalar=1.0 / IN, in1=brep_t[:],
                op0=MULT, op1=ADD,
            )
            OUT1 = squash(S1, 1)
            rep_mm, rep_cp = replicate(OUT1, 1)

            TMPk = TMP[:].rearrange("p (c n k) -> p c n k", c=CH, n=NSH)
            TMPc = TMP[:].rearrange("p (n k c) -> p n k c", n=NSH, k=DC)
            U1k = U1[:].rearrange("p (c n k) -> p c n k", c=CH, n=NSH)
            U2k = U2[:].rearrange("p (c n k) -> p c n k", c=CH, n=NSH)
            U1c = U1[:].rearrange("p (n k c) -> p n k c", n=NSH, k=DC)
            U2c = U2[:].rearrange("p (n k c) -> p n k c", n=NSH, k=DC)

            for it in (2, 3):
                # ---- a-step: TMP = IH * OUTr ; A = tree-fold k ----
                nc.vector.tensor_mul(
                    TMP[:].rearrange("p (c nk) -> p c nk", c=CH),
                    IH[:].rearrange("p c nk -> p c nk"),
                    OUTr[:].rearrange("p (o nk) -> p o nk", o=1)
                          .broadcast_to([128, CH, NK]),
                )
                nc.vector.tensor_add(U1k[:, :, :, 0:32], TMPk[:, :, :, 0:32],
                                     TMPk[:, :, :, 32:64])
                nc.vector.tensor_add(U2k[:, :, :, 0:16], U1k[:, :, :, 0:16],
                                     U1k[:, :, :, 16:32])
                nc.vector.tensor_add(U1k[:, :, :, 0:8], U2k[:, :, :, 0:8],
                                     U2k[:, :, :, 8:16])
                nc.vector.tensor_add(U2k[:, :, :, 0:4], U1k[:, :, :, 0:4],
                                     U1k[:, :, :, 4:8])
                nc.vector.tensor_add(U1k[:, :, :, 0:2], U2k[:, :, :, 0:2],
                                     U2k[:, :, :, 2:4])
                At = A2 if it == 2 else A3
                nc.vector.tensor_add(
                    At[:].rearrange("p (c n o) -> p c n o", c=CH, o=1),
                    U1k[:, :, :, 0:1], U1k[:, :, :, 1:2],
                )
                if it == 2:
                    BL = A2
                else:
                    BL = A3
                    nc.vector.tensor_add(A3[:], A3[:], A2[:])
                # ---- E = exp(BL), transposed write to [p, (n, c)] ----
                absorb("s", At[:2, :2])         # ACT clock <- tree (DVE)
                nc.scalar.activation(
                    E[:].rearrange("p (n c) -> p c n", n=NSH),
                    BL[:].rearrange("p (c n) -> p c n", c=CH),
                    AF.Exp,
                )
                # ---- Zp = sum_c E -> SCR[384:390] ----
                absorb("v", E[:2, :2])          # DVE clock <- E (ACT)
                with nc.allow_low_precision(reason="Z normalizer, positive sum"):
                    nc.vector.tensor_reduce(
                        SCR[:, NK:NK + NSH],
                        E[:].rearrange("p (n c) -> p n c", n=NSH),
                        axis=AX, op=ADD,
                    )
                # ---- s-step: TMP2 = IHC * E ; P2 = tree-fold c ----
                nc.vector.tensor_mul(
                    TMPc,
                    IHC[:, :, 0:CH]
                       .rearrange("p (n k) c -> p n k c", n=NSH),
                    E[:].rearrange("p (n o c) -> p n o c", n=NSH, o=1)
                       .broadcast_to([128, NSH, DC, CH]),
                )
                nc.vector.tensor_add(U1c[:, :, :, 0:16], TMPc[:, :, :, 0:16],
                                     TMPc[:, :, :, 16:32])
                nc.vector.tensor_add(U2c[:, :, :, 0:8], U1c[:, :, :, 0:8],
                                     U1c[:, :, :, 8:16])
                nc.vector.tensor_add(U1c[:, :, :, 0:4], U2c[:, :, :, 0:4],
                                     U2c[:, :, :, 4:8])
                nc.vector.tensor_add(U2c[:, :, :, 0:2], U1c[:, :, :, 0:2],
                                     U1c[:, :, :, 2:4])
                nc.vector.tensor_add(
                    SCR[:, 0:NK].rearrange("p (n k o) -> p n k o", n=NSH, o=1),
                    U2c[:, :, :, 0:1], U2c[:, :, :, 1:2])
                # ---- pS = BD4^T [P2|Zp] ----
                pS = ps_s.tile([B, NK + NSH], f32, tag="pS")
                mm_last = nc.tensor.matmul(pS[:], bd4_t[:], SCR[:],
                                           start=True, stop=True)
                mm_last_ref[0] = mm_last
                # ---- S = pS/Z + brep ----
                absorb("v", pS[:2, :2])         # DVE clock <- pS (PE)
                Rz = small.tile([B, NSH], f32, tag="Rz")
                nc.vector.reciprocal(Rz[:], pS[:, NK:NK + NSH])
                Su = work.tile([B, NK], f32, tag="Su2")
                nc.vector.tensor_mul(
                    Su[:].rearrange("p (n k) -> p n k", n=NSH),
                    pS[:, 0:NK].rearrange("p (n k) -> p n k", n=NSH),
                    Rz[:].rearrange("p (n o) -> p n o", o=1)
                        .broadcast_to([B, NSH, DC]),
                )
                S = work.tile([B, NK], f32, tag="S")
                nc.vector.tensor_add(S[:], Su[:], brep_t[:])
                OUT = squash(S, it)
                if it < 3:
                    rep_mm, rep_cp = replicate(OUT, it)
                else:
                    # absorb stream/cst DMA queue sems into SYNC first so the
                    # out-DMA's queue-reuse wait dedups to a single sem
                    for fin in (c_dma, *s_dmas):
                        fnop = nc.sync.nop()
                        add_dep_helper(fnop.ins, fin.ins, sync=True,
                                       reason="absorb DMA sem for queue reuse")
                    o_dma = nc.sync.dma_start(out=out_d[:], in_=OUT[:])
                    f_scr = small.tile([2, 4], f32, tag="fin")
                    f_act = nc.scalar.copy(f_scr[:, 0:2], OUT[:2, :2])
                    f_dve = nc.vector.tensor_copy(f_scr[:, 2:4], OUT[:2, :2])
                    for fin in (mm_last, f_act, f_dve, o_dma):
                        fnop = nc.sync.nop()
                        add_dep_helper(fnop.ins, fin.ins, sync=True,
                                       reason="absorb final sem for tail drain")

    return nc


def _pack_inputs(inputs, W, B_param):
    """Host-side shard + relayout. Returns list of 8 in_maps."""
    import ml_dtypes
    bf16 = ml_dtypes.bfloat16
    inputs = np.ascontiguousarray(inputs, dtype=np.float32)
    W = np.ascontiguousarray(W, dtype=np.float32)
    B_param = np.ascontiguousarray(B_param, dtype=np.float32)

    Wp = np.zeros((CH, NCP, DC, DIN), dtype=np.float32)
    Wp[:, :NC] = W
    Bp = np.zeros((NCP, DC), dtype=np.float32)
    Bp[:NC] = B_param

    # xt[(c,dc), dd, (b,rr)] = x[b, 4c+rr, 128dc+dd]
    x4 = inputs.reshape(B, CH, 4, 2, 128)           # b, c, rr, dc, dd
    xt = x4.transpose(1, 3, 4, 0, 2).reshape(CH * 2, 128, 128)
    bd4 = np.zeros((128, B), dtype=np.float32)
    bd4[np.arange(128), np.arange(128) // 4] = 1.0
    bd4t = bd4.T

    in_maps = []
    for core in range(NCORES):
        sl = slice(core * NSH, (core + 1) * NSH)
        Wc = Wp[:, sl]                               # c, n, k, d
        w5 = Wc.reshape(CH, NSH, DC, 2, 128)         # c n k dc dd
        wtc = w5.transpose(0, 3, 4, 1, 2).reshape(CH * 2, 128, NK)
        cstc = np.zeros((128, 544), dtype=np.float32)
        cstc[:, 0:B] = bd4
        cstc[0:B, B:B + 128] = bd4t
        cstc[0:B, B + 128:B + 128 + NK] = np.broadcast_to(
            Bp[sl].reshape(1, NK), (B, NK))
        xwc = np.concatenate([xt, wtc], axis=2)      # [64, 128, 512]
        # reorder chunks to the kernel's interleaved channel order
        ch_order = []
        for t in range(CH // 2):
            ch_order += [t, t + CH // 2]
        perm = np.zeros(CH * 2, dtype=np.int64)
        for p_, c_ in enumerate(ch_order):
            perm[2 * p_] = 2 * c_
            perm[2 * p_ + 1] = 2 * c_ + 1
        xwc = xwc[perm]
        # partition-major: [d, cd, 512] for 32KB-contiguous DMA runs
        xwc = np.ascontiguousarray(xwc.transpose(1, 0, 2)).astype(bf16)
        in_maps.append(dict(xw=xwc, cst=cstc.astype(bf16)))
    return in_maps


def _run(inputs, W, B_param, trace=False):
    from concourse.bass_utils import run_bass_kernel_spmd

    if "nc" not in _cache:
        _cache["nc"] = _build_nc()
    nc = _cache["nc"]
    in_maps = _pack_inputs(inputs, W, B_param)
    res = run_bass_kernel_spmd(nc, in_maps, core_ids=list(range(NCORES)),
                               trace=trace)
    outs = [r["out"].reshape(B, NSH, DC) for r in res.results]
    full = np.concatenate(outs, axis=1)[:, :NC, :]
    return np.ascontiguousarray(full.astype(np.float32)), res


def kernel(inputs, W, B_param):
    out, _ = _run(inputs, W, B_param, trace=False)
    return out


# revision 27
# speedup vs baseline: 1.0010x; 1.0010x over previous
"""Trainium2 Bass kernel for nn_CapsuleLayer (B=32, In=128, Din=256, ch=32, Nc=47, Dc=64).

Sharding: over the OUTPUT-CAPSULE axis Nc (47 -> pad 48 = 8 cores x 6 capsules).
W (94 MiB) is the dominant HBM tensor -- Nc-sharding reads W exactly once total.

bf16 pipeline (rel_err ~6e-3 vs 2e-2 gate):
- stream (x|W) in bf16, partition-major HBM layout -> 32KB-contiguous DMA runs
- inputs_hat via bf16 matmuls (1 cy/row vs fp32's 4)
- IH stored TWICE from PSUM: k-inner [p,(c,n,k)] for the a-step and c-inner
  [p,(n,k,c)] for the s-step, so both big DVE muls hit the 2x bf16 perf mode
  (packed innermost operands; measured 0.64 ns/col vs 1.28 broadcast/1x)
- reductions as pairwise bf16 tree-adds (2x) instead of TENSOR_REDUCE (1x)

Routing iteration t (per core, Nsh=6 capsules):
  TMP  = IH * OUTr            (DVE 2x, k-inner)
  A    = tree-fold k 64->1    (DVE 2x, last level fp32)
  E    = exp(sum_t A)         (ACT, written transposed to [p,(n,c)])
  Zp   = reduce_c E           (DVE, into SCRATCH[384:390])
  TMP2 = IHC * E              (DVE 2x, c-inner)
  P2   = tree-fold c 32->1    (DVE 2x, into SCRATCH[0:384])
  pS   = BD4^T [P2|Zp]        (PE partition reduce over (b,rr))
  S    = pS/Z + Brep ; OUT = squash(S)  (small [32,384] ops)
Iteration 1 (uniform c): S1 = psum_s1/IN + Brep via PSUM-accumulated
BD4^T IH_c matmuls during phase 1.

Toolchain constraint: EVERY engine instruction accepts at most ONE sync wait
at codegen.  Same-engine deps are free (program order / one monotonic sem per
engine); cross-engine fan-in is handled by absorb ops (tiny reads that
pre-observe a sem) and dummy matmuls on the PE.
"""

import numpy as np

B, IN, DIN = 32, 128, 256
CH, NC, DC = 32, 47, 64
NCP = 48          # padded Nc
NSH = 6           # capsules per core
NCORES = 8
NK = NSH * DC     # 384
EPS = 1e-7

_cache = {}


def _build_nc():
    import concourse.bass as bass
    import concourse.tile as tile
    from concourse import mybir
    from concourse.tile_rust import add_dep_helper

    f32 = mybir.dt.float32
    bf = mybir.dt.bfloat16
    nc = bass.Bass()

    # partition-major packed stream: xw[d, cd, 0:128]=xT, [128:512]=wT (bf16)
    xw = nc.dram_tensor("xw", [128, CH * 2, 512], bf, kind="ExternalInput")
    # consts: [bd4(0:32) | bd4t(rows0:32, 32:160) | brep(rows0:32, 160:544)]
    cst = nc.dram_tensor("cst", [128, 544], bf, kind="ExternalInput")
    out_d = nc.dram_tensor("out", [B, NK], f32, kind="ExternalOutput")

    ADD = mybir.AluOpType.add
    MULT = mybir.AluOpType.mult
    AX = mybir.AxisListType.X
    AF = mybir.ActivationFunctionType

    with tile.TileContext(nc) as tc:
        with (
            tc.tile_pool(name="singles", bufs=1) as singles,
            tc.tile_pool(name="work", bufs=1) as work,
            tc.tile_pool(name="small", bufs=2) as small,
            tc.tile_pool(name="ps_ih", bufs=3, space="PSUM") as ps_ih,
            tc.tile_pool(name="ps_s1", bufs=1, space="PSUM") as ps_s1,
            tc.tile_pool(name="ps_s", bufs=2, space="PSUM") as ps_s,
            tc.tile_pool(name="ps_rep", bufs=2, space="PSUM") as ps_rep,
        ):
            cst_t = singles.tile([128, 544], bf)
            c_dma = nc.sync.dma_start(out=cst_t[:], in_=cst[:])
            bd4_t = cst_t[:, 0:B]                 # [128, 32] bf16
            bd4t_t = cst_t[0:B, B:B + 128]        # [32, 128] bf16
            brep_t = cst_t[0:B, B + 128:B + 128 + NK]   # [32, 384] bf16
            eps_t = singles.tile([B, 1], f32)
            nc.vector.memset(eps_t[:], EPS)
            # DVE/ACT pre-observe the const-DMA sem
            dve_scratch = singles.tile([4, 8], bf)
            nc.vector.tensor_copy(dve_scratch[:2, 0:2], cst_t[:2, :2])
            act_scratch = singles.tile([4, 8], bf)
            nc.scalar.copy(act_scratch[:2, 0:2], cst_t[:2, :2])
            act_f32 = singles.tile([4, 2], f32)
            nc.scalar.activation(act_f32[:2, 0:2], act_scratch[:2, 0:2],
                                 AF.Exp)

            IH = singles.tile([128, CH, NK], bf)      # k-inner
            IHC = singles.tile([128, NK, CH + 1], bf)  # c-inner, pad stride 33
            STREAM = singles.tile([128, CH * 2, 512], bf)
            TMP = singles.tile([128, CH * NK], bf)    # mul product scratch
            U1 = singles.tile([128, 6144], bf)
            U2 = singles.tile([128, 3072], bf)
            SCR = singles.tile([128, NK + NSH], bf)   # [P2 | Zp]
            A2 = singles.tile([128, CH * NSH], f32)
            A3 = singles.tile([128, CH * NSH], f32)
            E = singles.tile([128, NSH * CH], bf)     # [p, (n, c)]
            OUTr = singles.tile([128, NK], bf)

            # Absorb the const-DMA sem into the PE clock (PE nop).
            last_dummy = nc.tensor.nop()
            add_dep_helper(last_dummy.ins, c_dma.ins, sync=True,
                           reason="absorb cst DMA sem into PE clock")

            # ---------------- phase 1: inputs_hat + iter-1 s ----------------
            s_dmas = []
            dma_splits = [(0, 2), (2, 22), (22, 43), (43, 64)]
            for gi, (lo, hi) in enumerate(dma_splits):
                dd = nc.sync.dma_start(
                    out=STREAM[:, lo:hi, :],
                    in_=xw[:, lo:hi, :],
                )
                if gi > 0:
                    add_dep_helper(dd.ins, s_dmas[0].ins, sync=True,
                                   reason="first chunk gets full DMA bandwidth")
                s_dmas.append(dd)
            # channel processing order (c, c+16) interleaved so the iter-1
            # tree-fold over c can start mid-phase (chunk j needs channels
            # 4j..4j+3 and 16+4j..19+4j = the first 8(j+1) positions)
            ch_order = []
            for t in range(CH // 2):
                ch_order += [t, t + CH // 2]
            U1s = U1[:].rearrange("p (n k c) -> p n k c", n=NSH, k=DC)

            copy_last = []      # last psum reader per position
            for pos, c in enumerate(ch_order):
                if pos >= 3:
                    # absorb the psum-slot WAR ticks into the PE clock
                    for cl_ins in copy_last[pos - 3]:
                        dmy = nc.tensor.nop()
                        add_dep_helper(dmy.ins, cl_ins.ins, sync=True,
                                       reason="absorb psum WAR tick on PE")
                        last_dummy = dmy
                psum_ih = ps_ih.tile([128, NK], f32, tag="ih")
                for dc in range(2):
                    cd = pos * 2 + dc
                    mih = nc.tensor.matmul(
                        psum_ih[:], STREAM[:, cd, 0:128], STREAM[:, cd, 128:512],
                        start=(dc == 0), stop=(dc == 1),
                    )
                    if dc == 0:
                        add_dep_helper(mih.ins, last_dummy.ins, sync=False,
                                       reason="order dummy before matmul")
                # IH (packed dst) on DVE: 0.56us; IHC (strided dst) on ACT:
                # 0.58us -- DVE runs strided casts at 1.8us, so never there
                cv = nc.vector.tensor_copy(IH[:, c, :], psum_ih[:])
                ca = nc.scalar.copy(IHC[:, :, c], psum_ih[:])
                # stagger: ACT reads the slot only after DVE is done with it,
                # so the two engines never contend on one PSUM bank
                add_dep_helper(ca.ins, cv.ins, sync=True,
                               reason="stagger psum readers")
                copy_last.append((cv, ca))

            _absn = [0]

            def absorb(eng, src_ap):
                """Tiny copy on `eng` reading src_ap: pre-observes the
                producer's sem so the next real op keeps a single wait."""
                _absn[0] += 1
                scr = small.tile([2, 2], f32, tag="abs%d" % _absn[0])
                if eng == "v":
                    return nc.vector.tensor_copy(scr[:], src_ap)
                return nc.scalar.copy(scr[:], src_ap)

            def squash(S, it):
                """S: [B, NK] f32 sbuf tile -> OUT tile (bf16 it<3, f32 it=3)."""
                Ssq = work.tile([B, NK], f32, tag="Su")
                nc.vector.tensor_mul(Ssq[:], S[:], S[:])
                m2 = small.tile([B, NSH], f32, tag="m2")
                nc.vector.tensor_reduce(
                    m2[:], Ssq[:].rearrange("p (n k) -> p n k", n=NSH),
                    axis=AX, op=ADD,
                )
                d1 = small.tile([B, NSH], f32, tag="d1")
                nc.vector.tensor_scalar_add(d1[:], m2[:], 1.0)
                rd1 = small.tile([B, NSH], f32, tag="rd1")
                nc.vector.reciprocal(rd1[:], d1[:])
                absorb("s", m2[:2, :2])          # ACT clock <- m2 (DVE)
                # rsqrt(m2+eps) = exp(-0.5*ln(m2+eps)); ln+exp share one
                # ACT table set (no SQRT table thrash)
                ln_ = small.tile([B, NSH], f32, tag="ln")
                nc.scalar.activation(ln_[:], m2[:], AF.Ln, bias=eps_t[:])
                rsq = small.tile([B, NSH], f32, tag="rsq")
                nc.scalar.activation(rsq[:], ln_[:], AF.Exp, scale=-0.5)
                absorb("v", rsq[:2, :2])         # DVE clock <- rsq (ACT)
                t_ = small.tile([B, NSH], f32, tag="t")
                nc.vector.tensor_mul(t_[:], m2[:], rsq[:])
                g_ = small.tile([B, NSH], f32, tag="g")
                nc.vector.tensor_mul(g_[:], t_[:], rd1[:])
                OUT = work.tile([B, NK], f32 if it == 3 else bf,
                                tag="out%d" % it)
                nc.vector.tensor_mul(
                    OUT[:].rearrange("p (n k) -> p n k", n=NSH),
                    S[:].rearrange("p (n k) -> p n k", n=NSH),
                    g_[:].rearrange("p (n o) -> p n o", o=1)
                        .broadcast_to([B, NSH, DC]),
                )
                return OUT

            rep_mm_prev = [None]
            mm_last_ref = [None]

            def replicate(OUTb, it):
                """OUTb [B, NK] bf16 -> OUTr [128, NK] bf16 (row b -> 4b..4b+3)."""
                pr = ps_rep.tile([128, NK], f32, tag="rep")
                mm = nc.tensor.matmul(pr[:], bd4t_t[:], OUTb[:],
                                      start=True, stop=True)
                rep_mm_prev[0] = mm
                cp = nc.vector.tensor_copy(OUTr[:], pr[:])
                return mm, cp

            # ---------------- iter 1 (uniform routing: E=1) ----------------
            # fold c 32->1 over IHC on DVE, pinned after the last copy so the
            # scheduler cannot interleave it into the copy stream
            for j in range(4):
                nc.vector.tensor_add(
                    U1s[:, :, :, 4 * j:4 * j + 4],
                    IHC[:, :, 4 * j:4 * j + 4],
                    IHC[:, :, CH // 2 + 4 * j:CH // 2 + 4 * j + 4],
                )
            U2s = U2[:].rearrange("p (n k c) -> p n k c", n=NSH, k=DC)
            nc.vector.tensor_add(U2s[:, :, :, 0:8], U1s[:, :, :, 0:8],
                                 U1s[:, :, :, 8:16])
            nc.vector.tensor_add(U1s[:, :, :, 0:4], U2s[:, :, :, 0:4],
                                 U2s[:, :, :, 4:8])
            nc.vector.tensor_add(U2s[:, :, :, 0:2], U1s[:, :, :, 0:2],
                                 U1s[:, :, :, 2:4])
            nc.vector.tensor_add(
                SCR[:, 0:NK].rearrange("p (n k o) -> p n k o", n=NSH, o=1),
                U2s[:, :, :, 0:1], U2s[:, :, :, 1:2])
            pS1 = ps_s1.tile([B, NK], f32)
            nc.tensor.matmul(pS1[:], bd4_t[:], SCR[:, 0:NK],
                             start=True, stop=True)
            S1 = work.tile([B, NK], f32, tag="S")
            nc.vector.scalar_tensor_tensor(
                out=S1[:], in0=pS1[:], scalar=1.0 / IN, in1=brep_t[:],
                op0=MULT, op1=ADD,
            )
            OUT1 = squash(S1, 1)
            rep_mm, rep_cp = replicate(OUT1, 1)

            TMPk = TMP[:].rearrange("p (c n k) -> p c n k", c=CH, n=NSH)
            TMPc = TMP[:].rearrange("p (n k c) -> p n k c", n=NSH, k=DC)
            U1k = U1[:].rearrange("p (c n k) -> p c n k", c=CH, n=NSH)
            U2k = U2[:].rearrange("p (c n k) -> p c n k", c=CH, n=NSH)
            U1c = U1[:].rearrange("p (n k c) -> p n k c", n=NSH, k=DC)
            U2c = U2[:].rearrange("p (n k c) -> p n k c", n=NSH, k=DC)

            for it in (2, 3):
                # ---- a-step: TMP = IH * OUTr ; A = tree-fold k ----
                nc.vector.tensor_mul(
                    TMP[:].rearrange("p (c nk) -> p c nk", c=CH),
                    IH[:].rearrange("p c nk -> p c nk"),
                    OUTr[:].rearrange("p (o nk) -> p o nk", o=1)
                          .broadcast_to([128, CH, NK]),
                )
                nc.vector.tensor_add(U1k[:, :, :, 0:32], TMPk[:, :, :, 0:32],
                                     TMPk[:, :, :, 32:64])
                nc.vector.tensor_add(U2k[:, :, :, 0:16], U1k[:, :, :, 0:16],
                                     U1k[:, :, :, 16:32])
                nc.vector.tensor_add(U1k[:, :, :, 0:8], U2k[:, :, :, 0:8],
                                     U2k[:, :, :, 8:16])
                nc.vector.tensor_add(U2k[:, :, :, 0:4], U1k[:, :, :, 0:4],
                                     U1k[:, :, :, 4:8])
                nc.vector.tensor_add(U1k[:, :, :, 0:2], U2k[:, :, :, 0:2],
                                     U2k[:, :, :, 2:4])
                At = A2 if it == 2 else A3
                nc.vector.tensor_add(
                    At[:].rearrange("p (c n o) -> p c n o", c=CH, o=1),
                    U1k[:, :, :, 0:1], U1k[:, :, :, 1:2],
                )
                if it == 2:
                    BL = A2
                else:
                    BL = A3
                    nc.vector.tensor_add(A3[:], A3[:], A2[:])
                # ---- E = exp(BL), transposed write to [p, (n, c)] ----
                absorb("s", At[:2, :2])         # ACT clock <- tree (DVE)
                nc.scalar.activation(
                    E[:].rearrange("p (n c) -> p c n", n=NSH),
                    BL[:].rearrange("p (c n) -> p c n", c=CH),
                    AF.Exp,
                )
                # ---- Zp = sum_c E -> SCR[384:390] ----
                absorb("v", E[:2, :2])          # DVE clock <- E (ACT)
                with nc.allow_low_precision(reason="Z normalizer, positive sum"):
                    nc.vector.tensor_reduce(
                        SCR[:, NK:NK + NSH],
                        E[:].rearrange("p (n c) -> p n c", n=NSH),
                        axis=AX, op=ADD,
                    )
                # ---- s-step: TMP2 = IHC * E ; P2 = tree-fold c ----
                nc.vector.tensor_mul(
                    TMPc,
                    IHC[:, :, 0:CH]
                       .rearrange("p (n k) c -> p n k c", n=NSH),
                    E[:].rearrange("p (n o c) -> p n o c", n=NSH, o=1)
                       .broadcast_to([128, NSH, DC, CH]),
                )
                nc.vector.tensor_add(U1c[:, :, :, 0:16], TMPc[:, :, :, 0:16],
                                     TMPc[:, :, :, 16:32])
                nc.vector.tensor_add(U2c[:, :, :, 0:8], U1c[:, :, :, 0:8],
                                     U1c[:, :, :, 8:16])
                nc.vector.tensor_add(U1c[:, :, :, 0:4], U2c[:, :, :, 0:4],
                                     U2c[:, :, :, 4:8])
                nc.vector.tensor_add(U2c[:, :, :, 0:2], U1c[:, :, :, 0:2],
                                     U1c[:, :, :, 2:4])
                nc.vector.tensor_add(
                    SCR[:, 0:NK].rearrange("p (n k o) -> p n k o", n=NSH, o=1),
                    U2c[:, :, :, 0:1], U2c[:, :, :, 1:2])
                # ---- pS = BD4^T [P2|Zp] ----
                pS = ps_s.tile([B, NK + NSH], f32, tag="pS")
                mm_last = nc.tensor.matmul(pS[:], bd4_t[:], SCR[:],
                                           start=True, stop=True)
                mm_last_ref[0] = mm_last
                # ---- S = pS/Z + brep ----
                absorb("v", pS[:2, :2])         # DVE clock <- pS (PE)
                Rz = small.tile([B, NSH], f32, tag="Rz")
                nc.vector.reciprocal(Rz[:], pS[:, NK:NK + NSH])
                Su = work.tile([B, NK], f32, tag="Su2")
                nc.vector.tensor_mul(
                    Su[:].rearrange("p (n k) -> p n k", n=NSH),
                    pS[:, 0:NK].rearrange("p (n k) -> p n k", n=NSH),
                    Rz[:].rearrange("p (n o) -> p n o", o=1)
                        .broadcast_to([B, NSH, DC]),
                )
                S = work.tile([B, NK], f32, tag="S")
                nc.vector.tensor_add(S[:], Su[:], brep_t[:])
                OUT = squash(S, it)
                if it < 3:
                    rep_mm, rep_cp = replicate(OUT, it)
                else:
                    # absorb stream/cst DMA queue sems into SYNC first so the
                    # out-DMA's queue-reuse wait dedups to a single sem
                    for fin in (c_dma, *s_dmas):
                        fnop = nc.sync.nop()
                        add_dep_helper(fnop.ins, fin.ins, sync=True,
                                       reason="absorb DMA sem for queue reuse")
                    o_dma = nc.sync.dma_start(out=out_d[:], in_=OUT[:])
                    f_scr = small.tile([2, 4], f32, tag="fin")
                    f_act = nc.scalar.copy(f_scr[:, 0:2], OUT[:2, :2])
                    f_dve = nc.vector.tensor_copy(f_scr[:, 2:4], OUT[:2, :2])
                    for fin in (mm_last, f_act, f_dve, o_dma):
                        fnop = nc.sync.nop()
                        add_dep_helper(fnop.ins, fin.ins, sync=True,
                                       reason="absorb final sem for tail drain")

    return nc


def _pack_inputs(inputs, W, B_param):
    """Host-side shard + relayout. Returns list of 8 in_maps."""
    import ml_dtypes
    bf16 = ml_dtypes.bfloat16
    inputs = np.ascontiguousarray(inputs, dtype=np.float32)
    W = np.ascontiguousarray(W, dtype=np.float32)
    B_param = np.ascontiguousarray(B_param, dtype=np.float32)

    Wp = np.zeros((CH, NCP, DC, DIN), dtype=np.float32)
    Wp[:, :NC] = W
    Bp = np.zeros((NCP, DC), dtype=np.float32)
    Bp[:NC] = B_param

    # xt[(c,dc), dd, (b,rr)] = x[b, 4c+rr, 128dc+dd]
    x4 = inputs.reshape(B, CH, 4, 2, 128)           # b, c, rr, dc, dd
    xt = x4.transpose(1, 3, 4, 0, 2).reshape(CH * 2, 128, 128)
    bd4 = np.zeros((128, B), dtype=np.float32)
    bd4[np.arange(128), np.arange(128) // 4] = 1.0
    bd4t = bd4.T

    in_maps = []
    for core in range(NCORES):
        sl = slice(core * NSH, (core + 1) * NSH)
        Wc = Wp[:, sl]                               # c, n, k, d
        w5 = Wc.reshape(CH, NSH, DC, 2, 128)         # c n k dc dd
        wtc = w5.transpose(0, 3, 4, 1, 2).reshape(CH * 2, 128, NK)
        cstc = np.zeros((128, 544), dtype=np.float32)
        cstc[:, 0:B] = bd4
        cstc[0:B, B:B + 128] = bd4t
        cstc[0:B, B + 128:B + 128 + NK] = np.broadcast_to(
            Bp[sl].reshape(1, NK), (B, NK))
        xwc = np.concatenate([xt, wtc], axis=2)      # [64, 128, 512]
        # reorder chunks to the kernel's interleaved channel order
        ch_order = []
        for t in range(CH // 2):
            ch_order += [t, t + CH // 2]
        perm = np.zeros(CH * 2, dtype=np.int64)
        for p_, c_ in enumerate(ch_order):
            perm[2 * p_] = 2 * c_
            perm[2 * p_ + 1] = 2 * c_ + 1
        xwc = xwc[perm]
        # partition-major: [d, cd, 512] for 32KB-contiguous DMA runs
        xwc = np.ascontiguousarray(xwc.transpose(1, 0, 2)).astype(bf16)
        in_maps.append(dict(xw=xwc, cst=cstc.astype(bf16)))
    return in_maps


def _run(inputs, W, B_param, trace=False):
    from concourse.bass_utils import run_bass_kernel_spmd

    if "nc" not in _cache:
        _cache["nc"] = _build_nc()
    nc = _cache["nc"]
    in_maps = _pack_inputs(inputs, W, B_param)
    res = run_bass_kernel_spmd(nc, in_maps, core_ids=list(range(NCORES)),
                               trace=trace)
    outs = [r["out"].reshape(B, NSH, DC) for r in res.results]
    full = np.concatenate(outs, axis=1)[:, :NC, :]
    return np.ascontiguousarray(full.astype(np.float32)), res


def kernel(inputs, W, B_param):
    out, _ = _run(inputs, W, B_param, trace=False)
    return out


# revision 28
# speedup vs baseline: 1.0079x; 1.0068x over previous
"""Trainium2 Bass kernel for nn_CapsuleLayer (B=32, In=128, Din=256, ch=32, Nc=47, Dc=64).

Sharding: over the OUTPUT-CAPSULE axis Nc (47 -> pad 48 = 8 cores x 6 capsules).
W (94 MiB) is the dominant HBM tensor -- Nc-sharding reads W exactly once total.

bf16 pipeline (rel_err ~6e-3 vs 2e-2 gate):
- stream (x|W) in bf16, partition-major HBM layout -> 32KB-contiguous DMA runs
- inputs_hat via bf16 matmuls (1 cy/row vs fp32's 4)
- IH stored TWICE from PSUM: k-inner [p,(c,n,k)] for the a-step and c-inner
  [p,(n,k,c)] for the s-step, so both big DVE muls hit the 2x bf16 perf mode
  (packed innermost operands; measured 0.64 ns/col vs 1.28 broadcast/1x)
- reductions as pairwise bf16 tree-adds (2x) instead of TENSOR_REDUCE (1x)

Routing iteration t (per core, Nsh=6 capsules):
  TMP  = IH * OUTr            (DVE 2x, k-inner)
  A    = tree-fold k 64->1    (DVE 2x, last level fp32)
  E    = exp(sum_t A)         (ACT, written transposed to [p,(n,c)])
  Zp   = reduce_c E           (DVE, into SCRATCH[384:390])
  TMP2 = IHC * E              (DVE 2x, c-inner)
  P2   = tree-fold c 32->1    (DVE 2x, into SCRATCH[0:384])
  pS   = BD4^T [P2|Zp]        (PE partition reduce over (b,rr))
  S    = pS/Z + Brep ; OUT = squash(S)  (small [32,384] ops)
Iteration 1 (uniform c): S1 = psum_s1/IN + Brep via PSUM-accumulated
BD4^T IH_c matmuls during phase 1.

Toolchain constraint: EVERY engine instruction accepts at most ONE sync wait
at codegen.  Same-engine deps are free (program order / one monotonic sem per
engine); cross-engine fan-in is handled by absorb ops (tiny reads that
pre-observe a sem) and dummy matmuls on the PE.
"""

import numpy as np

B, IN, DIN = 32, 128, 256
CH, NC, DC = 32, 47, 64
NCP = 48          # padded Nc
NSH = 6           # capsules per core
NCORES = 8
NK = NSH * DC     # 384
EPS = 1e-7

_cache = {}


def _build_nc():
    import concourse.bass as bass
    import concourse.tile as tile
    from concourse import mybir
    from concourse.tile_rust import add_dep_helper

    f32 = mybir.dt.float32
    bf = mybir.dt.bfloat16
    nc = bass.Bass()

    # partition-major packed stream: xw[d, cd, 0:128]=xT, [128:512]=wT (bf16)
    xw = nc.dram_tensor("xw", [128, CH * 2, 512], bf, kind="ExternalInput")
    # consts: [bd4(0:32) | bd4t(rows0:32, 32:160) | brep(rows0:32, 160:544)]
    cst = nc.dram_tensor("cst", [128, 544], bf, kind="ExternalInput")
    out_d = nc.dram_tensor("out", [B, NK], f32, kind="ExternalOutput")

    ADD = mybir.AluOpType.add
    MULT = mybir.AluOpType.mult
    AX = mybir.AxisListType.X
    AF = mybir.ActivationFunctionType

    with tile.TileContext(nc) as tc:
        with (
            tc.tile_pool(name="singles", bufs=1) as singles,
            tc.tile_pool(name="work", bufs=1) as work,
            tc.tile_pool(name="small", bufs=2) as small,
            tc.tile_pool(name="ps_ih", bufs=3, space="PSUM") as ps_ih,
            tc.tile_pool(name="ps_s1", bufs=1, space="PSUM") as ps_s1,
            tc.tile_pool(name="ps_s", bufs=2, space="PSUM") as ps_s,
            tc.tile_pool(name="ps_rep", bufs=2, space="PSUM") as ps_rep,
        ):
            cst_t = singles.tile([128, 544], bf)
            c_dma = nc.sync.dma_start(out=cst_t[:], in_=cst[:])
            bd4_t = cst_t[:, 0:B]                 # [128, 32] bf16
            bd4t_t = cst_t[0:B, B:B + 128]        # [32, 128] bf16
            brep_t = cst_t[0:B, B + 128:B + 128 + NK]   # [32, 384] bf16
            eps_t = singles.tile([B, 1], f32)
            nc.vector.memset(eps_t[:], EPS)
            # DVE/ACT pre-observe the const-DMA sem
            dve_scratch = singles.tile([4, 8], bf)
            nc.vector.tensor_copy(dve_scratch[:2, 0:2], cst_t[:2, :2])
            act_scratch = singles.tile([4, 8], bf)
            nc.scalar.copy(act_scratch[:2, 0:2], cst_t[:2, :2])
            act_f32 = singles.tile([4, 2], f32)
            nc.scalar.activation(act_f32[:2, 0:2], act_scratch[:2, 0:2],
                                 AF.Exp)

            IH = singles.tile([128, CH, NK], bf)      # k-inner
            IHC = singles.tile([128, NK, CH + 1], bf)  # c-inner, pad stride 33
            STREAM = singles.tile([128, CH * 2, 512], bf)
            TMP = singles.tile([128, CH * NK], bf)    # mul product scratch
            U1 = singles.tile([128, 6144], bf)
            U2 = singles.tile([128, 3072], bf)
            SCR = singles.tile([128, NK + NSH], bf)   # [P2 | Zp]
            A2 = singles.tile([128, CH * NSH], f32)
            A3 = singles.tile([128, CH * NSH], f32)
            E = singles.tile([128, NSH * CH], bf)     # [p, (n, c)]
            OUTr = singles.tile([128, NK], bf)

            # Absorb the const-DMA sem into the PE clock (PE nop).
            last_dummy = nc.tensor.nop()
            add_dep_helper(last_dummy.ins, c_dma.ins, sync=True,
                           reason="absorb cst DMA sem into PE clock")

            # ---------------- phase 1: inputs_hat + iter-1 s ----------------
            s_dmas = []
            dma_splits = [(0, 2), (2, 22), (22, 43), (43, 64)]
            for gi, (lo, hi) in enumerate(dma_splits):
                dd = nc.sync.dma_start(
                    out=STREAM[:, lo:hi, :],
                    in_=xw[:, lo:hi, :],
                )
                if gi > 0:
                    add_dep_helper(dd.ins, s_dmas[0].ins, sync=True,
                                   reason="first chunk gets full DMA bandwidth")
                s_dmas.append(dd)
            # channel processing order (c, c+16) interleaved so the iter-1
            # tree-fold over c can start mid-phase (chunk j needs channels
            # 4j..4j+3 and 16+4j..19+4j = the first 8(j+1) positions)
            ch_order = []
            for t in range(CH // 2):
                ch_order += [t, t + CH // 2]
            U1s = U1[:].rearrange("p (n k c) -> p n k c", n=NSH, k=DC)

            copy_last = []      # last psum reader per position
            for pos, c in enumerate(ch_order):
                if pos >= 3:
                    # absorb the psum-slot WAR ticks into the PE clock
                    for cl_ins in copy_last[pos - 3]:
                        dmy = nc.tensor.nop()
                        add_dep_helper(dmy.ins, cl_ins.ins, sync=True,
                                       reason="absorb psum WAR tick on PE")
                        last_dummy = dmy
                psum_ih = ps_ih.tile([128, NK], f32, tag="ih")
                for dc in range(2):
                    cd = pos * 2 + dc
                    mih = nc.tensor.matmul(
                        psum_ih[:], STREAM[:, cd, 0:128], STREAM[:, cd, 128:512],
                        start=(dc == 0), stop=(dc == 1),
                    )
                    if dc == 0:
                        add_dep_helper(mih.ins, last_dummy.ins, sync=False,
                                       reason="order dummy before matmul")
                # IH (packed dst) on DVE: 0.56us; IHC (strided dst) on ACT:
                # 0.58us -- DVE runs strided casts at 1.8us, so never there
                cv = nc.vector.tensor_copy(IH[:, c, :], psum_ih[:])
                # IHC transposed copy sources from SBUF (IH), not PSUM: only
                # cv holds the psum slot, and ACT reads a cheap packed src
                ca = nc.scalar.copy(IHC[:, :, c], IH[:, c, :])
                copy_last.append((cv,))

            _absn = [0]

            def absorb(eng, src_ap):
                """Tiny copy on `eng` reading src_ap: pre-observes the
                producer's sem so the next real op keeps a single wait."""
                _absn[0] += 1
                scr = small.tile([2, 2], f32, tag="abs%d" % _absn[0])
                if eng == "v":
                    return nc.vector.tensor_copy(scr[:], src_ap)
                return nc.scalar.copy(scr[:], src_ap)

            def squash(S, it):
                """S: [B, NK] f32 sbuf tile -> OUT tile (bf16 it<3, f32 it=3)."""
                Ssq = work.tile([B, NK], f32, tag="Su")
                nc.vector.tensor_mul(Ssq[:], S[:], S[:])
                m2 = small.tile([B, NSH], f32, tag="m2")
                nc.vector.tensor_reduce(
                    m2[:], Ssq[:].rearrange("p (n k) -> p n k", n=NSH),
                    axis=AX, op=ADD,
                )
                d1 = small.tile([B, NSH], f32, tag="d1")
                nc.vector.tensor_scalar_add(d1[:], m2[:], 1.0)
                rd1 = small.tile([B, NSH], f32, tag="rd1")
                nc.vector.reciprocal(rd1[:], d1[:])
                absorb("s", m2[:2, :2])          # ACT clock <- m2 (DVE)
                # rsqrt(m2+eps) = exp(-0.5*ln(m2+eps)); ln+exp share one
                # ACT table set (no SQRT table thrash)
                ln_ = small.tile([B, NSH], f32, tag="ln")
                nc.scalar.activation(ln_[:], m2[:], AF.Ln, bias=eps_t[:])
                rsq = small.tile([B, NSH], f32, tag="rsq")
                nc.scalar.activation(rsq[:], ln_[:], AF.Exp, scale=-0.5)
                absorb("v", rsq[:2, :2])         # DVE clock <- rsq (ACT)
                t_ = small.tile([B, NSH], f32, tag="t")
                nc.vector.tensor_mul(t_[:], m2[:], rsq[:])
                g_ = small.tile([B, NSH], f32, tag="g")
                nc.vector.tensor_mul(g_[:], t_[:], rd1[:])
                OUT = work.tile([B, NK], f32 if it == 3 else bf,
                                tag="out%d" % it)
                nc.vector.tensor_mul(
                    OUT[:].rearrange("p (n k) -> p n k", n=NSH),
                    S[:].rearrange("p (n k) -> p n k", n=NSH),
                    g_[:].rearrange("p (n o) -> p n o", o=1)
                        .broadcast_to([B, NSH, DC]),
                )
                return OUT

            rep_mm_prev = [None]
            mm_last_ref = [None]

            def replicate(OUTb, it):
                """OUTb [B, NK] bf16 -> OUTr [128, NK] bf16 (row b -> 4b..4b+3)."""
                pr = ps_rep.tile([128, NK], f32, tag="rep")
                mm = nc.tensor.matmul(pr[:], bd4t_t[:], OUTb[:],
                                      start=True, stop=True)
                rep_mm_prev[0] = mm
                cp = nc.vector.tensor_copy(OUTr[:], pr[:])
                return mm, cp

            # ---------------- iter 1 (uniform routing: E=1) ----------------
            # fold c 32->1 over IHC on DVE, pinned after the last copy so the
            # scheduler cannot interleave it into the copy stream
            for j in range(4):
                nc.vector.tensor_add(
                    U1s[:, :, :, 4 * j:4 * j + 4],
                    IHC[:, :, 4 * j:4 * j + 4],
                    IHC[:, :, CH // 2 + 4 * j:CH // 2 + 4 * j + 4],
                )
            U2s = U2[:].rearrange("p (n k c) -> p n k c", n=NSH, k=DC)
            nc.vector.tensor_add(U2s[:, :, :, 0:8], U1s[:, :, :, 0:8],
                                 U1s[:, :, :, 8:16])
            nc.vector.tensor_add(U1s[:, :, :, 0:4], U2s[:, :, :, 0:4],
                                 U2s[:, :, :, 4:8])
            nc.vector.tensor_add(U2s[:, :, :, 0:2], U1s[:, :, :, 0:2],
                                 U1s[:, :, :, 2:4])
            nc.vector.tensor_add(
                SCR[:, 0:NK].rearrange("p (n k o) -> p n k o", n=NSH, o=1),
                U2s[:, :, :, 0:1], U2s[:, :, :, 1:2])
            pS1 = ps_s1.tile([B, NK], f32)
            nc.tensor.matmul(pS1[:], bd4_t[:], SCR[:, 0:NK],
                             start=True, stop=True)
            S1 = work.tile([B, NK], f32, tag="S")
            nc.vector.scalar_tensor_tensor(
                out=S1[:], in0=pS1[:], scalar=1.0 / IN, in1=brep_t[:],
                op0=MULT, op1=ADD,
            )
            OUT1 = squash(S1, 1)
            rep_mm, rep_cp = replicate(OUT1, 1)

            TMPk = TMP[:].rearrange("p (c n k) -> p c n k", c=CH, n=NSH)
            TMPc = TMP[:].rearrange("p (n k c) -> p n k c", n=NSH, k=DC)
            U1k = U1[:].rearrange("p (c n k) -> p c n k", c=CH, n=NSH)
            U2k = U2[:].rearrange("p (c n k) -> p c n k", c=CH, n=NSH)
            U1c = U1[:].rearrange("p (n k c) -> p n k c", n=NSH, k=DC)
            U2c = U2[:].rearrange("p (n k c) -> p n k c", n=NSH, k=DC)

            for it in (2, 3):
                # ---- a-step: TMP = IH * OUTr ; A = tree-fold k ----
                nc.vector.tensor_mul(
                    TMP[:].rearrange("p (c nk) -> p c nk", c=CH),
                    IH[:].rearrange("p c nk -> p c nk"),
                    OUTr[:].rearrange("p (o nk) -> p o nk", o=1)
                          .broadcast_to([128, CH, NK]),
                )
                nc.vector.tensor_add(U1k[:, :, :, 0:32], TMPk[:, :, :, 0:32],
                                     TMPk[:, :, :, 32:64])
                nc.vector.tensor_add(U2k[:, :, :, 0:16], U1k[:, :, :, 0:16],
                                     U1k[:, :, :, 16:32])
                nc.vector.tensor_add(U1k[:, :, :, 0:8], U2k[:, :, :, 0:8],
                                     U2k[:, :, :, 8:16])
                nc.vector.tensor_add(U2k[:, :, :, 0:4], U1k[:, :, :, 0:4],
                                     U1k[:, :, :, 4:8])
                nc.vector.tensor_add(U1k[:, :, :, 0:2], U2k[:, :, :, 0:2],
                                     U2k[:, :, :, 2:4])
                At = A2 if it == 2 else A3
                nc.vector.tensor_add(
                    At[:].rearrange("p (c n o) -> p c n o", c=CH, o=1),
                    U1k[:, :, :, 0:1], U1k[:, :, :, 1:2],
                )
                if it == 2:
                    BL = A2
                else:
                    BL = A3
                    nc.vector.tensor_add(A3[:], A3[:], A2[:])
                # ---- E = exp(BL), transposed write to [p, (n, c)] ----
                absorb("s", At[:2, :2])         # ACT clock <- tree (DVE)
                nc.scalar.activation(
                    E[:].rearrange("p (n c) -> p c n", n=NSH),
                    BL[:].rearrange("p (c n) -> p c n", c=CH),
                    AF.Exp,
                )
                # ---- Zp = sum_c E -> SCR[384:390] ----
                absorb("v", E[:2, :2])          # DVE clock <- E (ACT)
                with nc.allow_low_precision(reason="Z normalizer, positive sum"):
                    nc.vector.tensor_reduce(
                        SCR[:, NK:NK + NSH],
                        E[:].rearrange("p (n c) -> p n c", n=NSH),
                        axis=AX, op=ADD,
                    )
                # ---- s-step: TMP2 = IHC * E ; P2 = tree-fold c ----
                nc.vector.tensor_mul(
                    TMPc,
                    IHC[:, :, 0:CH]
                       .rearrange("p (n k) c -> p n k c", n=NSH),
                    E[:].rearrange("p (n o c) -> p n o c", n=NSH, o=1)
                       .broadcast_to([128, NSH, DC, CH]),
                )
                nc.vector.tensor_add(U1c[:, :, :, 0:16], TMPc[:, :, :, 0:16],
                                     TMPc[:, :, :, 16:32])
                nc.vector.tensor_add(U2c[:, :, :, 0:8], U1c[:, :, :, 0:8],
                                     U1c[:, :, :, 8:16])
                nc.vector.tensor_add(U1c[:, :, :, 0:4], U2c[:, :, :, 0:4],
                                     U2c[:, :, :, 4:8])
                nc.vector.tensor_add(U2c[:, :, :, 0:2], U1c[:, :, :, 0:2],
                                     U1c[:, :, :, 2:4])
                nc.vector.tensor_add(
                    SCR[:, 0:NK].rearrange("p (n k o) -> p n k o", n=NSH, o=1),
                    U2c[:, :, :, 0:1], U2c[:, :, :, 1:2])
                # ---- pS = BD4^T [P2|Zp] ----
                pS = ps_s.tile([B, NK + NSH], f32, tag="pS")
                mm_last = nc.tensor.matmul(pS[:], bd4_t[:], SCR[:],
                                           start=True, stop=True)
                mm_last_ref[0] = mm_last
                # ---- S = pS/Z + brep ----
                absorb("v", pS[:2, :2])         # DVE clock <- pS (PE)
                Rz = small.tile([B, NSH], f32, tag="Rz")
                nc.vector.reciprocal(Rz[:], pS[:, NK:NK + NSH])
                Su = work.tile([B, NK], f32, tag="Su2")
                nc.vector.tensor_mul(
                    Su[:].rearrange("p (n k) -> p n k", n=NSH),
                    pS[:, 0:NK].rearrange("p (n k) -> p n k", n=NSH),
                    Rz[:].rearrange("p (n o) -> p n o", o=1)
                        .broadcast_to([B, NSH, DC]),
                )
                S = work.tile([B, NK], f32, tag="S")
                nc.vector.tensor_add(S[:], Su[:], brep_t[:])
                OUT = squash(S, it)
                if it < 3:
                    rep_mm, rep_cp = replicate(OUT, it)
                else:
                    # absorb stream/cst DMA queue sems into SYNC first so the
                    # out-DMA's queue-reuse wait dedups to a single sem
                    for fin in (c_dma, *s_dmas):
                        fnop = nc.sync.nop()
                        add_dep_helper(fnop.ins, fin.ins, sync=True,
                                       reason="absorb DMA sem for queue reuse")
                    o_dma = nc.sync.dma_start(out=out_d[:], in_=OUT[:])
                    f_scr = small.tile([2, 4], f32, tag="fin")
                    f_act = nc.scalar.copy(f_scr[:, 0:2], OUT[:2, :2])
                    f_dve = nc.vector.tensor_copy(f_scr[:, 2:4], OUT[:2, :2])
                    for fin in (mm_last, f_act, f_dve, o_dma):
                        fnop = nc.sync.nop()
                        add_dep_helper(fnop.ins, fin.ins, sync=True,
                                       reason="absorb final sem for tail drain")

    return nc


def _pack_inputs(inputs, W, B_param):
    """Host-side shard + relayout. Returns list of 8 in_maps."""
    import ml_dtypes
    bf16 = ml_dtypes.bfloat16
    inputs = np.ascontiguousarray(inputs, dtype=np.float32)
    W = np.ascontiguousarray(W, dtype=np.float32)
    B_param = np.ascontiguousarray(B_param, dtype=np.float32)

    Wp = np.zeros((CH, NCP, DC, DIN), dtype=np.float32)
    Wp[:, :NC] = W
    Bp = np.zeros((NCP, DC), dtype=np.float32)
    Bp[:NC] = B_param

    # xt[(c,dc), dd, (b,rr)] = x[b, 4c+rr, 128dc+dd]
    x4 = inputs.reshape(B, CH, 4, 2, 128)           # b, c, rr, dc, dd
    xt = x4.transpose(1, 3, 4, 0, 2).reshape(CH * 2, 128, 128)
    bd4 = np.zeros((128, B), dtype=np.float32)
    bd4[np.arange(128), np.arange(128) // 4] = 1.0
    bd4t = bd4.T

    in_maps = []
    for core in range(NCORES):
        sl = slice(core * NSH, (core + 1) * NSH)
        Wc = Wp[:, sl]                               # c, n, k, d
        w5 = Wc.reshape(CH, NSH, DC, 2, 128)         # c n k dc dd
        wtc = w5.transpose(0, 3, 4, 1, 2).reshape(CH * 2, 128, NK)
        cstc = np.zeros((128, 544), dtype=np.float32)
        cstc[:, 0:B] = bd4
        cstc[0:B, B:B + 128] = bd4t
        cstc[0:B, B + 128:B + 128 + NK] = np.broadcast_to(
            Bp[sl].reshape(1, NK), (B, NK))
        xwc = np.concatenate([xt, wtc], axis=2)      # [64, 128, 512]
        # reorder chunks to the kernel's interleaved channel order
        ch_order = []
        for t in range(CH // 2):
            ch_order += [t, t + CH // 2]
        perm = np.zeros(CH * 2, dtype=np.int64)
        for p_, c_ in enumerate(ch_order):
            perm[2 * p_] = 2 * c_
            perm[2 * p_ + 1] = 2 * c_ + 1
        xwc = xwc[perm]
        # partition-major: [d, cd, 512] for 32KB-contiguous DMA runs
        xwc = np.ascontiguousarray(xwc.transpose(1, 0, 2)).astype(bf16)
        in_maps.append(dict(xw=xwc, cst=cstc.astype(bf16)))
    return in_maps


def _run(inputs, W, B_param, trace=False):
    from concourse.bass_utils import run_bass_kernel_spmd

    if "nc" not in _cache:
        _cache["nc"] = _build_nc()
    nc = _cache["nc"]
    in_maps = _pack_inputs(inputs, W, B_param)
    res = run_bass_kernel_spmd(nc, in_maps, core_ids=list(range(NCORES)),
                               trace=trace)
    outs = [r["out"].reshape(B, NSH, DC) for r in res.results]
    full = np.concatenate(outs, axis=1)[:, :NC, :]
    return np.ascontiguousarray(full.astype(np.float32)), res


def kernel(inputs, W, B_param):
    out, _ = _run(inputs, W, B_param, trace=False)
    return out


# revision 29
# speedup vs baseline: 1.1867x; 1.1774x over previous
"""Trainium2 Bass kernel for nn_CapsuleLayer (B=32, In=128, Din=256, ch=32, Nc=47, Dc=64).

Sharding: over the OUTPUT-CAPSULE axis Nc (47 -> pad 48 = 8 cores x 6 capsules).
W (94 MiB) is the dominant HBM tensor -- Nc-sharding reads W exactly once total.

bf16 pipeline (rel_err ~6e-3 vs 2e-2 gate):
- stream (x|W) in bf16, partition-major HBM layout -> 32KB-contiguous DMA runs
- inputs_hat via bf16 matmuls (1 cy/row vs fp32's 4)
- IH stored TWICE from PSUM: k-inner [p,(c,n,k)] for the a-step and c-inner
  [p,(n,k,c)] for the s-step, so both big DVE muls hit the 2x bf16 perf mode
  (packed innermost operands; measured 0.64 ns/col vs 1.28 broadcast/1x)
- reductions as pairwise bf16 tree-adds (2x) instead of TENSOR_REDUCE (1x)

Routing iteration t (per core, Nsh=6 capsules):
  TMP  = IH * OUTr            (DVE 2x, k-inner)
  A    = tree-fold k 64->1    (DVE 2x, last level fp32)
  E    = exp(sum_t A)         (ACT, written transposed to [p,(n,c)])
  Zp   = reduce_c E           (DVE, into SCRATCH[384:390])
  TMP2 = IHC * E              (DVE 2x, c-inner)
  P2   = tree-fold c 32->1    (DVE 2x, into SCRATCH[0:384])
  pS   = BD4^T [P2|Zp]        (PE partition reduce over (b,rr))
  S    = pS/Z + Brep ; OUT = squash(S)  (small [32,384] ops)
Iteration 1 (uniform c): S1 = psum_s1/IN + Brep via PSUM-accumulated
BD4^T IH_c matmuls during phase 1.

Toolchain constraint: EVERY engine instruction accepts at most ONE sync wait
at codegen.  Same-engine deps are free (program order / one monotonic sem per
engine); cross-engine fan-in is handled by absorb ops (tiny reads that
pre-observe a sem) and dummy matmuls on the PE.
"""

import numpy as np

B, IN, DIN = 32, 128, 256
CH, NC, DC = 32, 47, 64
NCP = 48          # padded Nc
NSH = 6           # capsules per core
NCORES = 8
NK = NSH * DC     # 384
EPS = 1e-7

_cache = {}


def _build_nc():
    import concourse.bass as bass
    import concourse.tile as tile
    from concourse import mybir
    from concourse.tile_rust import add_dep_helper

    f32 = mybir.dt.float32
    bf = mybir.dt.bfloat16
    nc = bass.Bass()

    # partition-major packed stream: xw[d, cd, 0:128]=xT, [128:512]=wT (bf16)
    xw = nc.dram_tensor("xw", [128, CH * 2, 512], bf, kind="ExternalInput")
    # consts: [bd4(0:32) | bd4t(rows0:32, 32:160) | brep(rows0:32, 160:544)]
    cst = nc.dram_tensor("cst", [128, 544], bf, kind="ExternalInput")
    out_d = nc.dram_tensor("out", [B, NK], f32, kind="ExternalOutput")

    ADD = mybir.AluOpType.add
    MULT = mybir.AluOpType.mult
    AX = mybir.AxisListType.X
    AF = mybir.ActivationFunctionType

    with tile.TileContext(nc) as tc:
        with (
            tc.tile_pool(name="singles", bufs=1) as singles,
            tc.tile_pool(name="work", bufs=1) as work,
            tc.tile_pool(name="small", bufs=2) as small,
            tc.tile_pool(name="ps_ih", bufs=3, space="PSUM") as ps_ih,
            tc.tile_pool(name="ps_s1", bufs=1, space="PSUM") as ps_s1,
            tc.tile_pool(name="ps_s", bufs=2, space="PSUM") as ps_s,
            tc.tile_pool(name="ps_rep", bufs=2, space="PSUM") as ps_rep,
        ):
            cst_t = singles.tile([128, 544], bf)
            c_dma = nc.sync.dma_start(out=cst_t[:], in_=cst[:])
            bd4_t = cst_t[:, 0:B]                 # [128, 32] bf16
            bd4t_t = cst_t[0:B, B:B + 128]        # [32, 128] bf16
            brep_t = cst_t[0:B, B + 128:B + 128 + NK]   # [32, 384] bf16
            eps_t = singles.tile([B, 1], f32)
            nc.vector.memset(eps_t[:], EPS)
            # DVE/ACT pre-observe the const-DMA sem
            dve_scratch = singles.tile([4, 8], bf)
            nc.vector.tensor_copy(dve_scratch[:2, 0:2], cst_t[:2, :2])
            act_scratch = singles.tile([4, 8], bf)
            nc.scalar.copy(act_scratch[:2, 0:2], cst_t[:2, :2])
            act_f32 = singles.tile([4, 2], f32)
            nc.scalar.activation(act_f32[:2, 0:2], act_scratch[:2, 0:2],
                                 AF.Exp)

            IH = singles.tile([128, CH, NK], bf)      # k-inner
            IHC = singles.tile([128, NK, CH + 1], bf)  # c-inner, pad stride 33
            STREAM = singles.tile([128, CH * 2, 512], bf)
            TMP = singles.tile([128, CH * NK], bf)    # mul product scratch
            U1 = singles.tile([128, 6144], bf)
            U2 = singles.tile([128, 3072], bf)
            SCR = singles.tile([128, NK + NSH], bf)   # [P2 | Zp]
            A2 = singles.tile([128, CH * NSH], f32)
            A3 = singles.tile([128, CH * NSH], f32)
            E = singles.tile([128, NSH * CH], bf)     # [p, (n, c)]
            OUTr = singles.tile([128, NK], bf)

            # Absorb the const-DMA sem into the PE clock (PE nop).
            last_dummy = nc.tensor.nop()
            add_dep_helper(last_dummy.ins, c_dma.ins, sync=True,
                           reason="absorb cst DMA sem into PE clock")

            # ---------------- phase 1: inputs_hat + iter-1 s ----------------
            s_dmas = []
            dma_splits = [(0, 2), (2, 22), (22, 43), (43, 64)]
            for gi, (lo, hi) in enumerate(dma_splits):
                dd = nc.sync.dma_start(
                    out=STREAM[:, lo:hi, :],
                    in_=xw[:, lo:hi, :],
                )
                if gi > 0:
                    add_dep_helper(dd.ins, s_dmas[0].ins, sync=True,
                                   reason="first chunk gets full DMA bandwidth")
                s_dmas.append(dd)
            # channel processing order (c, c+16) interleaved so the iter-1
            # tree-fold over c can start mid-phase (chunk j needs channels
            # 4j..4j+3 and 16+4j..19+4j = the first 8(j+1) positions)
            ch_order = []
            for t in range(CH // 2):
                ch_order += [t, t + CH // 2]
            U1s = U1[:].rearrange("p (n k c) -> p n k c", n=NSH, k=DC)

            copy_last = []      # last psum reader per position
            for pos, c in enumerate(ch_order):
                if pos >= 3:
                    # absorb the psum-slot WAR ticks into the PE clock
                    for cl_ins in copy_last[pos - 3]:
                        dmy = nc.tensor.nop()
                        add_dep_helper(dmy.ins, cl_ins.ins, sync=True,
                                       reason="absorb psum WAR tick on PE")
                        last_dummy = dmy
                psum_ih = ps_ih.tile([128, NK], f32, tag="ih")
                for dc in range(2):
                    cd = pos * 2 + dc
                    mih = nc.tensor.matmul(
                        psum_ih[:], STREAM[:, cd, 0:128], STREAM[:, cd, 128:512],
                        start=(dc == 0), stop=(dc == 1),
                    )
                    if dc == 0:
                        add_dep_helper(mih.ins, last_dummy.ins, sync=False,
                                       reason="order dummy before matmul")
                # IH (packed dst) on DVE: 0.56us; IHC (strided dst) on ACT:
                # 0.58us -- DVE runs strided casts at 1.8us, so never there
                cv = nc.vector.tensor_copy(IH[:, c, :], psum_ih[:])
                # IHC transposed copies source from SBUF (IH) and are only
                # needed by iter-2's s-step: run them on ACT/gpsimd, off the
                # phase-1 critical path (only cv holds the psum slot)
                if pos % 3 == 2:
                    gcp = nc.gpsimd.tensor_copy(IHC[:, :, c], IH[:, c, :])
                    gp_ihc_last = gcp
                else:
                    nc.scalar.copy(IHC[:, :, c], IH[:, c, :])
                copy_last.append((cv,))

            _absn = [0]

            def absorb(eng, src_ap):
                """Tiny copy on `eng` reading src_ap: pre-observes the
                producer's sem so the next real op keeps a single wait."""
                _absn[0] += 1
                scr = small.tile([2, 2], f32, tag="abs%d" % _absn[0])
                if eng == "v":
                    return nc.vector.tensor_copy(scr[:], src_ap)
                return nc.scalar.copy(scr[:], src_ap)

            def squash(S, it):
                """S: [B, NK] f32 sbuf tile -> OUT tile (bf16 it<3, f32 it=3)."""
                Ssq = work.tile([B, NK], f32, tag="Su")
                nc.vector.tensor_mul(Ssq[:], S[:], S[:])
                m2 = small.tile([B, NSH], f32, tag="m2")
                nc.vector.tensor_reduce(
                    m2[:], Ssq[:].rearrange("p (n k) -> p n k", n=NSH),
                    axis=AX, op=ADD,
                )
                d1 = small.tile([B, NSH], f32, tag="d1")
                nc.vector.tensor_scalar_add(d1[:], m2[:], 1.0)
                rd1 = small.tile([B, NSH], f32, tag="rd1")
                nc.vector.reciprocal(rd1[:], d1[:])
                absorb("s", m2[:2, :2])          # ACT clock <- m2 (DVE)
                # rsqrt(m2+eps) = exp(-0.5*ln(m2+eps)); ln+exp share one
                # ACT table set (no SQRT table thrash)
                ln_ = small.tile([B, NSH], f32, tag="ln")
                nc.scalar.activation(ln_[:], m2[:], AF.Ln, bias=eps_t[:])
                rsq = small.tile([B, NSH], f32, tag="rsq")
                nc.scalar.activation(rsq[:], ln_[:], AF.Exp, scale=-0.5)
                absorb("v", rsq[:2, :2])         # DVE clock <- rsq (ACT)
                t_ = small.tile([B, NSH], f32, tag="t")
                nc.vector.tensor_mul(t_[:], m2[:], rsq[:])
                g_ = small.tile([B, NSH], f32, tag="g")
                nc.vector.tensor_mul(g_[:], t_[:], rd1[:])
                OUT = work.tile([B, NK], f32 if it == 3 else bf,
                                tag="out%d" % it)
                nc.vector.tensor_mul(
                    OUT[:].rearrange("p (n k) -> p n k", n=NSH),
                    S[:].rearrange("p (n k) -> p n k", n=NSH),
                    g_[:].rearrange("p (n o) -> p n o", o=1)
                        .broadcast_to([B, NSH, DC]),
                )
                return OUT

            rep_mm_prev = [None]
            mm_last_ref = [None]

            def replicate(OUTb, it):
                """OUTb [B, NK] bf16 -> OUTr [128, NK] bf16 (row b -> 4b..4b+3)."""
                pr = ps_rep.tile([128, NK], f32, tag="rep")
                mm = nc.tensor.matmul(pr[:], bd4t_t[:], OUTb[:],
                                      start=True, stop=True)
                rep_mm_prev[0] = mm
                cp = nc.vector.tensor_copy(OUTr[:], pr[:])
                return mm, cp

            # ---------------- iter 1 (uniform routing: E=1) ----------------
            # fold c 32->1 over IHC on DVE, pinned after the last copy so the
            # scheduler cannot interleave it into the copy stream
            U1f = U1[:].rearrange("p (c nk) -> p c nk", c=16)
            U2f = U2[:].rearrange("p (c nk) -> p c nk", c=8)
            IHf = IH[:].rearrange("p c nk -> p c nk")
            for j in range(4):
                nc.vector.tensor_add(
                    U1f[:, 4 * j:4 * j + 4, :],
                    IHf[:, 4 * j:4 * j + 4, :],
                    IHf[:, CH // 2 + 4 * j:CH // 2 + 4 * j + 4, :],
                )
            nc.vector.tensor_add(U2f[:], U1f[:, 0:8, :], U1f[:, 8:16, :])
            nc.vector.tensor_add(U1f[:, 0:4, :], U2f[:, 0:4, :],
                                 U2f[:, 4:8, :])
            nc.vector.tensor_add(U2f[:, 0:2, :], U1f[:, 0:2, :],
                                 U1f[:, 2:4, :])
            nc.vector.tensor_add(
                SCR[:, 0:NK].rearrange("p (o nk) -> p o nk", o=1),
                U2f[:, 0:1, :], U2f[:, 1:2, :])
            pS1 = ps_s1.tile([B, NK], f32)
            nc.tensor.matmul(pS1[:], bd4_t[:], SCR[:, 0:NK],
                             start=True, stop=True)
            S1 = work.tile([B, NK], f32, tag="S")
            nc.vector.scalar_tensor_tensor(
                out=S1[:], in0=pS1[:], scalar=1.0 / IN, in1=brep_t[:],
                op0=MULT, op1=ADD,
            )
            OUT1 = squash(S1, 1)
            rep_mm, rep_cp = replicate(OUT1, 1)

            TMPk = TMP[:].rearrange("p (c n k) -> p c n k", c=CH, n=NSH)
            TMPc = TMP[:].rearrange("p (n k c) -> p n k c", n=NSH, k=DC)
            U1k = U1[:].rearrange("p (c n k) -> p c n k", c=CH, n=NSH)
            U2k = U2[:].rearrange("p (c n k) -> p c n k", c=CH, n=NSH)
            U1c = U1[:].rearrange("p (n k c) -> p n k c", n=NSH, k=DC)
            U2c = U2[:].rearrange("p (n k c) -> p n k c", n=NSH, k=DC)

            for it in (2, 3):
                # ---- a-step: TMP = IH * OUTr ; A = tree-fold k ----
                nc.vector.tensor_mul(
                    TMP[:].rearrange("p (c nk) -> p c nk", c=CH),
                    IH[:].rearrange("p c nk -> p c nk"),
                    OUTr[:].rearrange("p (o nk) -> p o nk", o=1)
                          .broadcast_to([128, CH, NK]),
                )
                nc.vector.tensor_add(U1k[:, :, :, 0:32], TMPk[:, :, :, 0:32],
                                     TMPk[:, :, :, 32:64])
                nc.vector.tensor_add(U2k[:, :, :, 0:16], U1k[:, :, :, 0:16],
                                     U1k[:, :, :, 16:32])
                nc.vector.tensor_add(U1k[:, :, :, 0:8], U2k[:, :, :, 0:8],
                                     U2k[:, :, :, 8:16])
                nc.vector.tensor_add(U2k[:, :, :, 0:4], U1k[:, :, :, 0:4],
                                     U1k[:, :, :, 4:8])
                nc.vector.tensor_add(U1k[:, :, :, 0:2], U2k[:, :, :, 0:2],
                                     U2k[:, :, :, 2:4])
                At = A2 if it == 2 else A3
                nc.vector.tensor_add(
                    At[:].rearrange("p (c n o) -> p c n o", c=CH, o=1),
                    U1k[:, :, :, 0:1], U1k[:, :, :, 1:2],
                )
                if it == 2:
                    BL = A2
                else:
                    BL = A3
                    nc.vector.tensor_add(A3[:], A3[:], A2[:])
                # ---- E = exp(BL), transposed write to [p, (n, c)] ----
                absorb("s", At[:2, :2])         # ACT clock <- tree (DVE)
                nc.scalar.activation(
                    E[:].rearrange("p (n c) -> p c n", n=NSH),
                    BL[:].rearrange("p (c n) -> p c n", c=CH),
                    AF.Exp,
                )
                # ---- Zp = sum_c E -> SCR[384:390] ----
                absorb("v", E[:2, :2])          # DVE clock <- E (ACT)
                with nc.allow_low_precision(reason="Z normalizer, positive sum"):
                    nc.vector.tensor_reduce(
                        SCR[:, NK:NK + NSH],
                        E[:].rearrange("p (n c) -> p n c", n=NSH),
                        axis=AX, op=ADD,
                    )
                # ---- s-step: TMP2 = IHC * E ; P2 = tree-fold c ----
                if it == 2:
                    gsc2 = small.tile([2, 2], bf, tag="gihc")
                    ga2 = nc.vector.tensor_copy(gsc2[:], IHC[:2, :2, ch_order[2]])
                    add_dep_helper(ga2.ins, gp_ihc_last.ins, sync=True,
                                   reason="DVE clock <- gp IHC copies")
                m2v = nc.vector.tensor_mul(
                    TMPc,
                    IHC[:, :, 0:CH]
                       .rearrange("p (n k) c -> p n k c", n=NSH),
                    E[:].rearrange("p (n o c) -> p n o c", n=NSH, o=1)
                       .broadcast_to([128, NSH, DC, CH]),
                )
                if it == 2:
                    add_dep_helper(m2v.ins, ga2.ins, sync=False,
                                   reason="gp absorb before mul2")
                nc.vector.tensor_add(U1c[:, :, :, 0:16], TMPc[:, :, :, 0:16],
                                     TMPc[:, :, :, 16:32])
                nc.vector.tensor_add(U2c[:, :, :, 0:8], U1c[:, :, :, 0:8],
                                     U1c[:, :, :, 8:16])
                nc.vector.tensor_add(U1c[:, :, :, 0:4], U2c[:, :, :, 0:4],
                                     U2c[:, :, :, 4:8])
                nc.vector.tensor_add(U2c[:, :, :, 0:2], U1c[:, :, :, 0:2],
                                     U1c[:, :, :, 2:4])
                nc.vector.tensor_add(
                    SCR[:, 0:NK].rearrange("p (n k o) -> p n k o", n=NSH, o=1),
                    U2c[:, :, :, 0:1], U2c[:, :, :, 1:2])
                # ---- pS = BD4^T [P2|Zp] ----
                pS = ps_s.tile([B, NK + NSH], f32, tag="pS")
                mm_last = nc.tensor.matmul(pS[:], bd4_t[:], SCR[:],
                                           start=True, stop=True)
                mm_last_ref[0] = mm_last
                # ---- S = pS/Z + brep ----
                absorb("v", pS[:2, :2])         # DVE clock <- pS (PE)
                Rz = small.tile([B, NSH], f32, tag="Rz")
                nc.vector.reciprocal(Rz[:], pS[:, NK:NK + NSH])
                Su = work.tile([B, NK], f32, tag="Su2")
                nc.vector.tensor_mul(
                    Su[:].rearrange("p (n k) -> p n k", n=NSH),
                    pS[:, 0:NK].rearrange("p (n k) -> p n k", n=NSH),
                    Rz[:].rearrange("p (n o) -> p n o", o=1)
                        .broadcast_to([B, NSH, DC]),
                )
                S = work.tile([B, NK], f32, tag="S")
                nc.vector.tensor_add(S[:], Su[:], brep_t[:])
                OUT = squash(S, it)
                if it < 3:
                    rep_mm, rep_cp = replicate(OUT, it)
                else:
                    # absorb stream/cst DMA queue sems into SYNC first so the
                    # out-DMA's queue-reuse wait dedups to a single sem
                    for fin in (c_dma, *s_dmas):
                        fnop = nc.sync.nop()
                        add_dep_helper(fnop.ins, fin.ins, sync=True,
                                       reason="absorb DMA sem for queue reuse")
                    o_dma = nc.sync.dma_start(out=out_d[:], in_=OUT[:])
                    f_scr = small.tile([2, 4], f32, tag="fin")
                    f_act = nc.scalar.copy(f_scr[:, 0:2], OUT[:2, :2])
                    f_dve = nc.vector.tensor_copy(f_scr[:, 2:4], OUT[:2, :2])
                    for fin in (mm_last, f_act, f_dve, o_dma):
                        fnop = nc.sync.nop()
                        add_dep_helper(fnop.ins, fin.ins, sync=True,
                                       reason="absorb final sem for tail drain")

    return nc


def _pack_inputs(inputs, W, B_param):
    """Host-side shard + relayout. Returns list of 8 in_maps."""
    import ml_dtypes
    bf16 = ml_dtypes.bfloat16
    inputs = np.ascontiguousarray(inputs, dtype=np.float32)
    W = np.ascontiguousarray(W, dtype=np.float32)
    B_param = np.ascontiguousarray(B_param, dtype=np.float32)

    Wp = np.zeros((CH, NCP, DC, DIN), dtype=np.float32)
    Wp[:, :NC] = W
    Bp = np.zeros((NCP, DC), dtype=np.float32)
    Bp[:NC] = B_param

    # xt[(c,dc), dd, (b,rr)] = x[b, 4c+rr, 128dc+dd]
    x4 = inputs.reshape(B, CH, 4, 2, 128)           # b, c, rr, dc, dd
    xt = x4.transpose(1, 3, 4, 0, 2).reshape(CH * 2, 128, 128)
    bd4 = np.zeros((128, B), dtype=np.float32)
    bd4[np.arange(128), np.arange(128) // 4] = 1.0
    bd4t = bd4.T

    in_maps = []
    for core in range(NCORES):
        sl = slice(core * NSH, (core + 1) * NSH)
        Wc = Wp[:, sl]                               # c, n, k, d
        w5 = Wc.reshape(CH, NSH, DC, 2, 128)         # c n k dc dd
        wtc = w5.transpose(0, 3, 4, 1, 2).reshape(CH * 2, 128, NK)
        cstc = np.zeros((128, 544), dtype=np.float32)
        cstc[:, 0:B] = bd4
        cstc[0:B, B:B + 128] = bd4t
        cstc[0:B, B + 128:B + 128 + NK] = np.broadcast_to(
            Bp[sl].reshape(1, NK), (B, NK))
        xwc = np.concatenate([xt, wtc], axis=2)      # [64, 128, 512]
        # reorder chunks to the kernel's interleaved channel order
        ch_order = []
        for t in range(CH // 2):
            ch_order += [t, t + CH // 2]
        perm = np.zeros(CH * 2, dtype=np.int64)
        for p_, c_ in enumerate(ch_order):
            perm[2 * p_] = 2 * c_
            perm[2 * p_ + 1] = 2 * c_ + 1
        xwc = xwc[perm]
        # partition-major: [d, cd, 512] for 32KB-contiguous DMA runs
        xwc = np.ascontiguousarray(xwc.transpose(1, 0, 2)).astype(bf16)
        in_maps.append(dict(xw=xwc, cst=cstc.astype(bf16)))
    return in_maps


def _run(inputs, W, B_param, trace=False):
    from concourse.bass_utils import run_bass_kernel_spmd

    if "nc" not in _cache:
        _cache["nc"] = _build_nc()
    nc = _cache["nc"]
    in_maps = _pack_inputs(inputs, W, B_param)
    res = run_bass_kernel_spmd(nc, in_maps, core_ids=list(range(NCORES)),
                               trace=trace)
    outs = [r["out"].reshape(B, NSH, DC) for r in res.results]
    full = np.concatenate(outs, axis=1)[:, :NC, :]
    return np.ascontiguousarray(full.astype(np.float32)), res


def kernel(inputs, W, B_param):
    out, _ = _run(inputs, W, B_param, trace=False)
    return out


# revision 31
# speedup vs baseline: 1.1995x; 1.0109x over previous
"""Trainium2 Bass kernel for nn_CapsuleLayer (B=32, In=128, Din=256, ch=32, Nc=47, Dc=64).

Sharding: over the OUTPUT-CAPSULE axis Nc (47 -> pad 48 = 8 cores x 6 capsules).
W (94 MiB) is the dominant HBM tensor -- Nc-sharding reads W exactly once total.

bf16 pipeline (rel_err ~6e-3 vs 2e-2 gate):
- stream (x|W) in bf16, partition-major HBM layout -> 32KB-contiguous DMA runs
- inputs_hat via bf16 matmuls (1 cy/row vs fp32's 4)
- IH stored TWICE from PSUM: k-inner [p,(c,n,k)] for the a-step and c-inner
  [p,(n,k,c)] for the s-step, so both big DVE muls hit the 2x bf16 perf mode
  (packed innermost operands; measured 0.64 ns/col vs 1.28 broadcast/1x)
- reductions as pairwise bf16 tree-adds (2x) instead of TENSOR_REDUCE (1x)

Routing iteration t (per core, Nsh=6 capsules):
  TMP  = IH * OUTr            (DVE 2x, k-inner)
  A    = tree-fold k 64->1    (DVE 2x, last level fp32)
  E    = exp(sum_t A)         (ACT, written transposed to [p,(n,c)])
  Zp   = reduce_c E           (DVE, into SCRATCH[384:390])
  TMP2 = IHC * E              (DVE 2x, c-inner)
  P2   = tree-fold c 32->1    (DVE 2x, into SCRATCH[0:384])
  pS   = BD4^T [P2|Zp]        (PE partition reduce over (b,rr))
  S    = pS/Z + Brep ; OUT = squash(S)  (small [32,384] ops)
Iteration 1 (uniform c): S1 = psum_s1/IN + Brep via PSUM-accumulated
BD4^T IH_c matmuls during phase 1.

Toolchain constraint: EVERY engine instruction accepts at most ONE sync wait
at codegen.  Same-engine deps are free (program order / one monotonic sem per
engine); cross-engine fan-in is handled by absorb ops (tiny reads that
pre-observe a sem) and dummy matmuls on the PE.
"""

import numpy as np

B, IN, DIN = 32, 128, 256
CH, NC, DC = 32, 47, 64
NCP = 48          # padded Nc
NSH = 6           # capsules per core
NCORES = 8
NK = NSH * DC     # 384
EPS = 1e-7

_cache = {}


def _build_nc():
    import concourse.bass as bass
    import concourse.tile as tile
    from concourse import mybir
    from concourse.tile_rust import add_dep_helper

    f32 = mybir.dt.float32
    bf = mybir.dt.bfloat16
    nc = bass.Bass()

    # partition-major packed stream: xw[d, cd, 0:128]=xT, [128:512]=wT (bf16)
    xw = nc.dram_tensor("xw", [128, CH * 2, 512], bf, kind="ExternalInput")
    # consts: [bd4(0:32) | bd4t(rows0:32, 32:160) | brep(rows0:32, 160:544)]
    cst = nc.dram_tensor("cst", [128, 544], bf, kind="ExternalInput")
    out_d = nc.dram_tensor("out", [B, NK], f32, kind="ExternalOutput")

    ADD = mybir.AluOpType.add
    MULT = mybir.AluOpType.mult
    AX = mybir.AxisListType.X
    AF = mybir.ActivationFunctionType

    with tile.TileContext(nc) as tc:
        with (
            tc.tile_pool(name="singles", bufs=1) as singles,
            tc.tile_pool(name="work", bufs=1) as work,
            tc.tile_pool(name="small", bufs=2) as small,
            tc.tile_pool(name="ps_ih", bufs=3, space="PSUM") as ps_ih,
            tc.tile_pool(name="ps_s1", bufs=1, space="PSUM") as ps_s1,
            tc.tile_pool(name="ps_s", bufs=2, space="PSUM") as ps_s,
            tc.tile_pool(name="ps_rep", bufs=2, space="PSUM") as ps_rep,
        ):
            cst_t = singles.tile([128, 544], bf)
            bd4_t = cst_t[:, 0:B]                 # [128, 32] bf16
            bd4t_t = cst_t[0:B, B:B + 128]        # [32, 128] bf16
            brep_t = cst_t[0:B, B + 128:B + 128 + NK]   # [32, 384] bf16
            eps_t = singles.tile([B, 1], f32)
            nc.vector.memset(eps_t[:], EPS)

            IH = singles.tile([128, CH, NK], bf)      # k-inner
            IHC = singles.tile([128, NK, CH + 1], bf)  # c-inner, pad stride 33
            STREAM = singles.tile([128, CH * 2, 512], bf)
            TMP = singles.tile([128, CH * NK], bf)    # mul product scratch
            U1 = singles.tile([128, 6144], bf)
            U2 = singles.tile([128, 3072], bf)
            SCR = singles.tile([128, NK + NSH], bf)   # [P2 | Zp]
            A2 = singles.tile([128, CH * NSH], f32)
            A3 = singles.tile([128, CH * NSH], f32)
            E = singles.tile([128, NSH * CH], bf)     # [p, (n, c)]
            OUTr = singles.tile([128, NK], bf)

            # ---------------- phase 1: inputs_hat + iter-1 s ----------------
            s_dmas = []
            dma_splits = [(0, 2), (2, 22), (22, 43), (43, 64)]
            for gi, (lo, hi) in enumerate(dma_splits):
                dd = nc.sync.dma_start(
                    out=STREAM[:, lo:hi, :],
                    in_=xw[:, lo:hi, :],
                )
                if gi == 0:
                    # cst rides behind the first (small) stream chunk
                    c_dma = nc.sync.dma_start(out=cst_t[:], in_=cst[:])
                else:
                    add_dep_helper(dd.ins, s_dmas[0].ins, sync=True,
                                   reason="first chunk gets full DMA bandwidth")
                s_dmas.append(dd)
            # channel processing order (c, c+16) interleaved so the iter-1
            # tree-fold over c can start mid-phase (chunk j needs channels
            # 4j..4j+3 and 16+4j..19+4j = the first 8(j+1) positions)
            ch_order = []
            for t in range(CH // 2):
                ch_order += [t, t + CH // 2]
            U1s = U1[:].rearrange("p (n k c) -> p n k c", n=NSH, k=DC)

            # Absorb the const-DMA sem into the PE clock (PE nop).
            last_dummy = nc.tensor.nop()
            add_dep_helper(last_dummy.ins, c_dma.ins, sync=True,
                           reason="absorb cst DMA sem into PE clock")
            # DVE/ACT pre-observe the const-DMA sem
            dve_scratch = singles.tile([4, 8], bf)
            nc.vector.tensor_copy(dve_scratch[:2, 0:2], cst_t[:2, :2])
            act_scratch = singles.tile([4, 8], bf)
            nc.scalar.copy(act_scratch[:2, 0:2], cst_t[:2, :2])
            act_f32 = singles.tile([4, 2], f32)
            nc.scalar.activation(act_f32[:2, 0:2], act_scratch[:2, 0:2],
                                 AF.Exp)

            copy_last = []      # last psum reader per position
            for pos, c in enumerate(ch_order):
                if pos >= 3:
                    # absorb the psum-slot WAR ticks into the PE clock
                    for cl_ins in copy_last[pos - 3]:
                        dmy = nc.tensor.nop()
                        add_dep_helper(dmy.ins, cl_ins.ins, sync=True,
                                       reason="absorb psum WAR tick on PE")
                        last_dummy = dmy
                psum_ih = ps_ih.tile([128, NK], f32, tag="ih")
                for dc in range(2):
                    cd = pos * 2 + dc
                    mih = nc.tensor.matmul(
                        psum_ih[:], STREAM[:, cd, 0:128], STREAM[:, cd, 128:512],
                        start=(dc == 0), stop=(dc == 1),
                    )
                    if dc == 0:
                        add_dep_helper(mih.ins, last_dummy.ins, sync=False,
                                       reason="order dummy before matmul")
                # IH (packed dst) on DVE: 0.56us; IHC (strided dst) on ACT:
                # 0.58us -- DVE runs strided casts at 1.8us, so never there
                cv = nc.vector.tensor_copy(IH[:, c, :], psum_ih[:])
                # IHC transposed copies source from SBUF (IH) and are only
                # needed by iter-2's s-step: run them on ACT/gpsimd, off the
                # phase-1 critical path (only cv holds the psum slot)
                if pos % 3 == 2:
                    gcp = nc.gpsimd.tensor_copy(IHC[:, :, c], IH[:, c, :])
                    gp_ihc_last = gcp
                else:
                    nc.scalar.copy(IHC[:, :, c], IH[:, c, :])
                copy_last.append((cv,))

            _absn = [0]

            def absorb(eng, src_ap):
                """Tiny copy on `eng` reading src_ap: pre-observes the
                producer's sem so the next real op keeps a single wait."""
                _absn[0] += 1
                scr = small.tile([2, 2], f32, tag="abs%d" % _absn[0])
                if eng == "v":
                    return nc.vector.tensor_copy(scr[:], src_ap)
                return nc.scalar.copy(scr[:], src_ap)

            def squash(S, it):
                """S: [B, NK] f32 sbuf tile -> OUT tile (bf16 it<3, f32 it=3)."""
                Ssq = work.tile([B, NK], f32, tag="Su")
                nc.vector.tensor_mul(Ssq[:], S[:], S[:])
                m2 = small.tile([B, NSH], f32, tag="m2")
                nc.vector.tensor_reduce(
                    m2[:], Ssq[:].rearrange("p (n k) -> p n k", n=NSH),
                    axis=AX, op=ADD,
                )
                d1 = small.tile([B, NSH], f32, tag="d1")
                nc.vector.tensor_scalar_add(d1[:], m2[:], 1.0)
                rd1 = small.tile([B, NSH], f32, tag="rd1")
                nc.vector.reciprocal(rd1[:], d1[:])
                absorb("s", m2[:2, :2])          # ACT clock <- m2 (DVE)
                # rsqrt(m2+eps) = exp(-0.5*ln(m2+eps)); ln+exp share one
                # ACT table set (no SQRT table thrash)
                ln_ = small.tile([B, NSH], f32, tag="ln")
                nc.scalar.activation(ln_[:], m2[:], AF.Ln, bias=eps_t[:])
                rsq = small.tile([B, NSH], f32, tag="rsq")
                nc.scalar.activation(rsq[:], ln_[:], AF.Exp, scale=-0.5)
                absorb("v", rsq[:2, :2])         # DVE clock <- rsq (ACT)
                t_ = small.tile([B, NSH], f32, tag="t")
                nc.vector.tensor_mul(t_[:], m2[:], rsq[:])
                g_ = small.tile([B, NSH], f32, tag="g")
                nc.vector.tensor_mul(g_[:], t_[:], rd1[:])
                OUT = work.tile([B, NK], f32 if it == 3 else bf,
                                tag="out%d" % it)
                nc.vector.tensor_mul(
                    OUT[:].rearrange("p (n k) -> p n k", n=NSH),
                    S[:].rearrange("p (n k) -> p n k", n=NSH),
                    g_[:].rearrange("p (n o) -> p n o", o=1)
                        .broadcast_to([B, NSH, DC]),
                )
                return OUT

            rep_mm_prev = [None]
            mm_last_ref = [None]

            def replicate(OUTb, it):
                """OUTb [B, NK] bf16 -> OUTr [128, NK] bf16 (row b -> 4b..4b+3)."""
                pr = ps_rep.tile([128, NK], f32, tag="rep")
                mm = nc.tensor.matmul(pr[:], bd4t_t[:], OUTb[:],
                                      start=True, stop=True)
                rep_mm_prev[0] = mm
                cp = nc.vector.tensor_copy(OUTr[:], pr[:])
                return mm, cp

            # ---------------- iter 1 (uniform routing: E=1) ----------------
            # fold c 32->1 over IHC on DVE, pinned after the last copy so the
            # scheduler cannot interleave it into the copy stream
            U1f = U1[:].rearrange("p (c nk) -> p c nk", c=16)
            U2f = U2[:].rearrange("p (c nk) -> p c nk", c=8)
            IHf = IH[:].rearrange("p c nk -> p c nk")
            for j in range(4):
                nc.vector.tensor_add(
                    U1f[:, 4 * j:4 * j + 4, :],
                    IHf[:, 4 * j:4 * j + 4, :],
                    IHf[:, CH // 2 + 4 * j:CH // 2 + 4 * j + 4, :],
                )
                if j == 1:
                    # chunks 0+1 ready at position 15: hoists into the loop
                    nc.vector.tensor_add(U2f[:, 0:4, :], U1f[:, 0:4, :],
                                         U1f[:, 4:8, :])
            nc.vector.tensor_add(U2f[:, 4:8, :], U1f[:, 8:12, :],
                                 U1f[:, 12:16, :])
            nc.vector.tensor_add(U1f[:, 0:4, :], U2f[:, 0:4, :],
                                 U2f[:, 4:8, :])
            nc.vector.tensor_add(U2f[:, 0:2, :], U1f[:, 0:2, :],
                                 U1f[:, 2:4, :])
            nc.vector.tensor_add(
                SCR[:, 0:NK].rearrange("p (o nk) -> p o nk", o=1),
                U2f[:, 0:1, :], U2f[:, 1:2, :])
            pS1 = ps_s1.tile([B, NK], f32)
            nc.tensor.matmul(pS1[:], bd4_t[:], SCR[:, 0:NK],
                             start=True, stop=True)
            S1 = work.tile([B, NK], f32, tag="S")
            nc.vector.scalar_tensor_tensor(
                out=S1[:], in0=pS1[:], scalar=1.0 / IN, in1=brep_t[:],
                op0=MULT, op1=ADD,
            )
            OUT1 = squash(S1, 1)
            rep_mm, rep_cp = replicate(OUT1, 1)

            TMPk = TMP[:].rearrange("p (c n k) -> p c n k", c=CH, n=NSH)
            TMPc = TMP[:].rearrange("p (n k c) -> p n k c", n=NSH, k=DC)
            U1k = U1[:].rearrange("p (c n k) -> p c n k", c=CH, n=NSH)
            U2k = U2[:].rearrange("p (c n k) -> p c n k", c=CH, n=NSH)
            U1c = U1[:].rearrange("p (n k c) -> p n k c", n=NSH, k=DC)
            U2c = U2[:].rearrange("p (n k c) -> p n k c", n=NSH, k=DC)

            for it in (2, 3):
                # ---- a-step: TMP = IH * OUTr ; A = tree-fold k ----
                nc.vector.tensor_mul(
                    TMP[:].rearrange("p (c nk) -> p c nk", c=CH),
                    IH[:].rearrange("p c nk -> p c nk"),
                    OUTr[:].rearrange("p (o nk) -> p o nk", o=1)
                          .broadcast_to([128, CH, NK]),
                )
                nc.vector.tensor_add(U1k[:, :, :, 0:32], TMPk[:, :, :, 0:32],
                                     TMPk[:, :, :, 32:64])
                nc.vector.tensor_add(U2k[:, :, :, 0:16], U1k[:, :, :, 0:16],
                                     U1k[:, :, :, 16:32])
                nc.vector.tensor_add(U1k[:, :, :, 0:8], U2k[:, :, :, 0:8],
                                     U2k[:, :, :, 8:16])
                nc.vector.tensor_add(U2k[:, :, :, 0:4], U1k[:, :, :, 0:4],
                                     U1k[:, :, :, 4:8])
                nc.vector.tensor_add(U1k[:, :, :, 0:2], U2k[:, :, :, 0:2],
                                     U2k[:, :, :, 2:4])
                At = A2 if it == 2 else A3
                nc.vector.tensor_add(
                    At[:].rearrange("p (c n o) -> p c n o", c=CH, o=1),
                    U1k[:, :, :, 0:1], U1k[:, :, :, 1:2],
                )
                if it == 2:
                    BL = A2
                else:
                    BL = A3
                    nc.vector.tensor_add(A3[:], A3[:], A2[:])
                # ---- E = exp(BL), transposed write to [p, (n, c)] ----
                absorb("s", At[:2, :2])         # ACT clock <- tree (DVE)
                nc.scalar.activation(
                    E[:].rearrange("p (n c) -> p c n", n=NSH),
                    BL[:].rearrange("p (c n) -> p c n", c=CH),
                    AF.Exp,
                )
                # ---- Zp = sum_c E -> SCR[384:390] ----
                absorb("v", E[:2, :2])          # DVE clock <- E (ACT)
                with nc.allow_low_precision(reason="Z normalizer, positive sum"):
                    nc.vector.tensor_reduce(
                        SCR[:, NK:NK + NSH],
                        E[:].rearrange("p (n c) -> p n c", n=NSH),
                        axis=AX, op=ADD,
                    )
                # ---- s-step: TMP2 = IHC * E ; P2 = tree-fold c ----
                if it == 2:
                    gsc2 = small.tile([2, 2], bf, tag="gihc")
                    ga2 = nc.vector.tensor_copy(gsc2[:], IHC[:2, :2, ch_order[2]])
                    add_dep_helper(ga2.ins, gp_ihc_last.ins, sync=True,
                                   reason="DVE clock <- gp IHC copies")
                m2v = nc.vector.tensor_mul(
                    TMPc,
                    IHC[:, :, 0:CH]
                       .rearrange("p (n k) c -> p n k c", n=NSH),
                    E[:].rearrange("p (n o c) -> p n o c", n=NSH, o=1)
                       .broadcast_to([128, NSH, DC, CH]),
                )
                if it == 2:
                    add_dep_helper(m2v.ins, ga2.ins, sync=False,
                                   reason="gp absorb before mul2")
                nc.vector.tensor_add(U1c[:, :, :, 0:16], TMPc[:, :, :, 0:16],
                                     TMPc[:, :, :, 16:32])
                nc.vector.tensor_add(U2c[:, :, :, 0:8], U1c[:, :, :, 0:8],
                                     U1c[:, :, :, 8:16])
                nc.vector.tensor_add(U1c[:, :, :, 0:4], U2c[:, :, :, 0:4],
                                     U2c[:, :, :, 4:8])
                nc.vector.tensor_add(U2c[:, :, :, 0:2], U1c[:, :, :, 0:2],
                                     U1c[:, :, :, 2:4])
                nc.vector.tensor_add(
                    SCR[:, 0:NK].rearrange("p (n k o) -> p n k o", n=NSH, o=1),
                    U2c[:, :, :, 0:1], U2c[:, :, :, 1:2])
                # ---- pS = BD4^T [P2|Zp] ----
                pS = ps_s.tile([B, NK + NSH], f32, tag="pS")
                mm_last = nc.tensor.matmul(pS[:], bd4_t[:], SCR[:],
                                           start=True, stop=True)
                mm_last_ref[0] = mm_last
                # ---- S = pS/Z + brep ----
                absorb("v", pS[:2, :2])         # DVE clock <- pS (PE)
                Rz = small.tile([B, NSH], f32, tag="Rz")
                nc.vector.reciprocal(Rz[:], pS[:, NK:NK + NSH])
                Su = work.tile([B, NK], f32, tag="Su2")
                nc.vector.tensor_mul(
                    Su[:].rearrange("p (n k) -> p n k", n=NSH),
                    pS[:, 0:NK].rearrange("p (n k) -> p n k", n=NSH),
                    Rz[:].rearrange("p (n o) -> p n o", o=1)
                        .broadcast_to([B, NSH, DC]),
                )
                S = work.tile([B, NK], f32, tag="S")
                nc.vector.tensor_add(S[:], Su[:], brep_t[:])
                OUT = squash(S, it)
                if it < 3:
                    rep_mm, rep_cp = replicate(OUT, it)
                else:
                    # absorb stream/cst DMA queue sems into SYNC first so the
                    # out-DMA's queue-reuse wait dedups to a single sem
                    for fin in (c_dma, *s_dmas):
                        fnop = nc.sync.nop()
                        add_dep_helper(fnop.ins, fin.ins, sync=True,
                                       reason="absorb DMA sem for queue reuse")
                    o_dma = nc.sync.dma_start(out=out_d[:], in_=OUT[:])
                    f_scr = small.tile([2, 4], f32, tag="fin")
                    f_act = nc.scalar.copy(f_scr[:, 0:2], OUT[:2, :2])
                    f_dve = nc.vector.tensor_copy(f_scr[:, 2:4], OUT[:2, :2])
                    for fin in (mm_last, f_act, f_dve, o_dma):
                        fnop = nc.sync.nop()
                        add_dep_helper(fnop.ins, fin.ins, sync=True,
                                       reason="absorb final sem for tail drain")

    return nc


def _pack_inputs(inputs, W, B_param):
    """Host-side shard + relayout. Returns list of 8 in_maps."""
    import ml_dtypes
    bf16 = ml_dtypes.bfloat16
    inputs = np.ascontiguousarray(inputs, dtype=np.float32)
    W = np.ascontiguousarray(W, dtype=np.float32)
    B_param = np.ascontiguousarray(B_param, dtype=np.float32)

    Wp = np.zeros((CH, NCP, DC, DIN), dtype=np.float32)
    Wp[:, :NC] = W
    Bp = np.zeros((NCP, DC), dtype=np.float32)
    Bp[:NC] = B_param

    # xt[(c,dc), dd, (b,rr)] = x[b, 4c+rr, 128dc+dd]
    x4 = inputs.reshape(B, CH, 4, 2, 128)           # b, c, rr, dc, dd
    xt = x4.transpose(1, 3, 4, 0, 2).reshape(CH * 2, 128, 128)
    bd4 = np.zeros((128, B), dtype=np.float32)
    bd4[np.arange(128), np.arange(128) // 4] = 1.0
    bd4t = bd4.T

    in_maps = []
    for core in range(NCORES):
        sl = slice(core * NSH, (core + 1) * NSH)
        Wc = Wp[:, sl]                               # c, n, k, d
        w5 = Wc.reshape(CH, NSH, DC, 2, 128)         # c n k dc dd
        wtc = w5.transpose(0, 3, 4, 1, 2).reshape(CH * 2, 128, NK)
        cstc = np.zeros((128, 544), dtype=np.float32)
        cstc[:, 0:B] = bd4
        cstc[0:B, B:B + 128] = bd4t
        cstc[0:B, B + 128:B + 128 + NK] = np.broadcast_to(
            Bp[sl].reshape(1, NK), (B, NK))
        xwc = np.concatenate([xt, wtc], axis=2)      # [64, 128, 512]
        # reorder chunks to the kernel's interleaved channel order
        ch_order = []
        for t in range(CH // 2):
            ch_order += [t, t + CH // 2]
        perm = np.zeros(CH * 2, dtype=np.int64)
        for p_, c_ in enumerate(ch_order):
            perm[2 * p_] = 2 * c_
            perm[2 * p_ + 1] = 2 * c_ + 1
        xwc = xwc[perm]
        # partition-major: [d, cd, 512] for 32KB-contiguous DMA runs
        xwc = np.ascontiguousarray(xwc.transpose(1, 0, 2)).astype(bf16)
        in_maps.append(dict(xw=xwc, cst=cstc.astype(bf16)))
    return in_maps


def _run(inputs, W, B_param, trace=False):
    from concourse.bass_utils import run_bass_kernel_spmd

    if "nc" not in _cache:
        _cache["nc"] = _build_nc()
    nc = _cache["nc"]
    in_maps = _pack_inputs(inputs, W, B_param)
    res = run_bass_kernel_spmd(nc, in_maps, core_ids=list(range(NCORES)),
                               trace=trace)
    outs = [r["out"].reshape(B, NSH, DC) for r in res.results]
    full = np.concatenate(outs, axis=1)[:, :NC, :]
    return np.ascontiguousarray(full.astype(np.float32)), res


def kernel(inputs, W, B_param):
    out, _ = _run(inputs, W, B_param, trace=False)
    return out


# revision 32
# speedup vs baseline: 1.1997x; 1.0001x over previous
"""Trainium2 Bass kernel for nn_CapsuleLayer (B=32, In=128, Din=256, ch=32, Nc=47, Dc=64).

Sharding: over the OUTPUT-CAPSULE axis Nc (47 -> pad 48 = 8 cores x 6 capsules).
W (94 MiB) is the dominant HBM tensor -- Nc-sharding reads W exactly once total.

bf16 pipeline (rel_err ~6e-3 vs 2e-2 gate):
- stream (x|W) in bf16, partition-major HBM layout -> 32KB-contiguous DMA runs
- inputs_hat via bf16 matmuls (1 cy/row vs fp32's 4)
- IH stored TWICE from PSUM: k-inner [p,(c,n,k)] for the a-step and c-inner
  [p,(n,k,c)] for the s-step, so both big DVE muls hit the 2x bf16 perf mode
  (packed innermost operands; measured 0.64 ns/col vs 1.28 broadcast/1x)
- reductions as pairwise bf16 tree-adds (2x) instead of TENSOR_REDUCE (1x)

Routing iteration t (per core, Nsh=6 capsules):
  TMP  = IH * OUTr            (DVE 2x, k-inner)
  A    = tree-fold k 64->1    (DVE 2x, last level fp32)
  E    = exp(sum_t A)         (ACT, written transposed to [p,(n,c)])
  Zp   = reduce_c E           (DVE, into SCRATCH[384:390])
  TMP2 = IHC * E              (DVE 2x, c-inner)
  P2   = tree-fold c 32->1    (DVE 2x, into SCRATCH[0:384])
  pS   = BD4^T [P2|Zp]        (PE partition reduce over (b,rr))
  S    = pS/Z + Brep ; OUT = squash(S)  (small [32,384] ops)
Iteration 1 (uniform c): S1 = psum_s1/IN + Brep via PSUM-accumulated
BD4^T IH_c matmuls during phase 1.

Toolchain constraint: EVERY engine instruction accepts at most ONE sync wait
at codegen.  Same-engine deps are free (program order / one monotonic sem per
engine); cross-engine fan-in is handled by absorb ops (tiny reads that
pre-observe a sem) and dummy matmuls on the PE.
"""

import numpy as np

B, IN, DIN = 32, 128, 256
CH, NC, DC = 32, 47, 64
NCP = 48          # padded Nc
NSH = 6           # capsules per core
NCORES = 8
NK = NSH * DC     # 384
EPS = 1e-7

_cache = {}


def _build_nc():
    import concourse.bass as bass
    import concourse.tile as tile
    from concourse import mybir
    from concourse.tile_rust import add_dep_helper

    f32 = mybir.dt.float32
    bf = mybir.dt.bfloat16
    nc = bass.Bass()

    # partition-major packed stream: xw[d, cd, 0:128]=xT, [128:512]=wT (bf16)
    xw = nc.dram_tensor("xw", [128, CH * 2, 512], bf, kind="ExternalInput")
    # consts: [bd4(0:32) | bd4t(rows0:32, 32:160) | brep(rows0:32, 160:544)]
    cst = nc.dram_tensor("cst", [128, 544], bf, kind="ExternalInput")
    out_d = nc.dram_tensor("out", [B, NK], f32, kind="ExternalOutput")

    ADD = mybir.AluOpType.add
    MULT = mybir.AluOpType.mult
    AX = mybir.AxisListType.X
    AF = mybir.ActivationFunctionType

    with tile.TileContext(nc) as tc:
        with (
            tc.tile_pool(name="singles", bufs=1) as singles,
            tc.tile_pool(name="work", bufs=1) as work,
            tc.tile_pool(name="small", bufs=2) as small,
            tc.tile_pool(name="ps_ih", bufs=3, space="PSUM") as ps_ih,
            tc.tile_pool(name="ps_s1", bufs=1, space="PSUM") as ps_s1,
            tc.tile_pool(name="ps_s", bufs=2, space="PSUM") as ps_s,
            tc.tile_pool(name="ps_rep", bufs=2, space="PSUM") as ps_rep,
        ):
            cst_t = singles.tile([128, 544], bf)
            bd4_t = cst_t[:, 0:B]                 # [128, 32] bf16
            bd4t_t = cst_t[0:B, B:B + 128]        # [32, 128] bf16
            brep_t = cst_t[0:B, B + 128:B + 128 + NK]   # [32, 384] bf16
            eps_t = singles.tile([B, 1], f32)
            nc.vector.memset(eps_t[:], EPS)

            IH = singles.tile([128, CH, NK], bf)      # k-inner
            IHC = singles.tile([128, NK, CH + 1], bf)  # c-inner, pad stride 33
            STREAM = singles.tile([128, CH * 2, 512], bf)
            TMP = singles.tile([128, CH * NK], bf)    # mul product scratch
            U1 = singles.tile([128, 6144], bf)
            U2 = singles.tile([128, 3072], bf)
            SCR = singles.tile([128, NK + NSH], bf)   # [P2 | Zp]
            A2 = singles.tile([128, CH * NSH], f32)
            A3 = singles.tile([128, CH * NSH], f32)
            E = singles.tile([128, NSH * CH], bf)     # [p, (n, c)]
            OUTr = singles.tile([128, NK], bf)

            # ---------------- phase 1: inputs_hat + iter-1 s ----------------
            s_dmas = []
            dma_splits = [(0, 2), (2, 22), (22, 43), (43, 64)]
            for gi, (lo, hi) in enumerate(dma_splits):
                dd = nc.sync.dma_start(
                    out=STREAM[:, lo:hi, :],
                    in_=xw[:, lo:hi, :],
                )
                if gi == 0:
                    # cst rides behind the first (small) stream chunk
                    c_dma = nc.sync.dma_start(out=cst_t[:], in_=cst[:])
                else:
                    add_dep_helper(dd.ins, s_dmas[0].ins, sync=True,
                                   reason="first chunk gets full DMA bandwidth")
                s_dmas.append(dd)
            # channel processing order (c, c+16) interleaved so the iter-1
            # tree-fold over c can start mid-phase (chunk j needs channels
            # 4j..4j+3 and 16+4j..19+4j = the first 8(j+1) positions)
            ch_order = []
            for t in range(CH // 2):
                ch_order += [t, t + CH // 2]
            U1s = U1[:].rearrange("p (n k c) -> p n k c", n=NSH, k=DC)

            # Absorb the const-DMA sem into the PE clock (PE nop).
            last_dummy = nc.tensor.nop()
            add_dep_helper(last_dummy.ins, c_dma.ins, sync=True,
                           reason="absorb cst DMA sem into PE clock")
            # DVE/ACT pre-observe the const-DMA sem
            dve_scratch = singles.tile([4, 8], bf)
            nc.vector.tensor_copy(dve_scratch[:2, 0:2], cst_t[:2, :2])
            act_scratch = singles.tile([4, 8], bf)
            nc.scalar.copy(act_scratch[:2, 0:2], cst_t[:2, :2])
            act_f32 = singles.tile([4, 2], f32)
            nc.scalar.activation(act_f32[:2, 0:2], act_scratch[:2, 0:2],
                                 AF.Exp)

            copy_last = []      # last psum reader per position
            for pos, c in enumerate(ch_order):
                if pos >= 3:
                    # absorb the psum-slot WAR ticks into the PE clock
                    for cl_ins in copy_last[pos - 3]:
                        dmy = nc.tensor.nop()
                        add_dep_helper(dmy.ins, cl_ins.ins, sync=True,
                                       reason="absorb psum WAR tick on PE")
                        last_dummy = dmy
                psum_ih = ps_ih.tile([128, NK], f32, tag="ih")
                for dc in range(2):
                    cd = pos * 2 + dc
                    mih = nc.tensor.matmul(
                        psum_ih[:], STREAM[:, cd, 0:128], STREAM[:, cd, 128:512],
                        start=(dc == 0), stop=(dc == 1),
                    )
                    if dc == 0:
                        add_dep_helper(mih.ins, last_dummy.ins, sync=False,
                                       reason="order dummy before matmul")
                # IH (packed dst) on DVE: 0.56us; IHC (strided dst) on ACT:
                # 0.58us -- DVE runs strided casts at 1.8us, so never there
                cv = nc.vector.tensor_copy(IH[:, c, :], psum_ih[:])
                # IHC transposed copies source from SBUF (IH) and are only
                # needed by iter-2's s-step: run them on ACT/gpsimd, off the
                # phase-1 critical path (only cv holds the psum slot)
                if pos % 3 == 2:
                    gcp = nc.gpsimd.tensor_copy(IHC[:, :, c], IH[:, c, :])
                    gp_ihc_last = gcp
                else:
                    nc.scalar.copy(IHC[:, :, c], IH[:, c, :])
                copy_last.append((cv,))

            _absn = [0]

            def absorb(eng, src_ap):
                """Tiny copy on `eng` reading src_ap: pre-observes the
                producer's sem so the next real op keeps a single wait."""
                _absn[0] += 1
                scr = small.tile([2, 2], f32, tag="abs%d" % _absn[0])
                if eng == "v":
                    return nc.vector.tensor_copy(scr[:], src_ap)
                return nc.scalar.copy(scr[:], src_ap)

            def squash(S, it):
                """S: [B, NK] f32 sbuf tile -> OUT tile (bf16 it<3, f32 it=3)."""
                Ssq = work.tile([B, NK], f32, tag="Su")
                nc.vector.tensor_mul(Ssq[:], S[:], S[:])
                m2 = small.tile([B, NSH], f32, tag="m2")
                nc.vector.tensor_reduce(
                    m2[:], Ssq[:].rearrange("p (n k) -> p n k", n=NSH),
                    axis=AX, op=ADD,
                )
                d1 = small.tile([B, NSH], f32, tag="d1")
                nc.vector.tensor_scalar_add(d1[:], m2[:], 1.0)
                rd1 = small.tile([B, NSH], f32, tag="rd1")
                nc.vector.reciprocal(rd1[:], d1[:])
                absorb("s", m2[:2, :2])          # ACT clock <- m2 (DVE)
                # rsqrt(m2+eps) = exp(-0.5*ln(m2+eps)); ln+exp share one
                # ACT table set (no SQRT table thrash)
                ln_ = small.tile([B, NSH], f32, tag="ln")
                nc.scalar.activation(ln_[:], m2[:], AF.Ln, bias=eps_t[:])
                rsq = small.tile([B, NSH], f32, tag="rsq")
                nc.scalar.activation(rsq[:], ln_[:], AF.Exp, scale=-0.5)
                absorb("v", rsq[:2, :2])         # DVE clock <- rsq (ACT)
                t_ = small.tile([B, NSH], f32, tag="t")
                nc.vector.tensor_mul(t_[:], m2[:], rsq[:])
                g_ = small.tile([B, NSH], f32, tag="g")
                nc.vector.tensor_mul(g_[:], t_[:], rd1[:])
                OUT = work.tile([B, NK], f32 if it == 3 else bf,
                                tag="out%d" % it)
                nc.vector.tensor_mul(
                    OUT[:].rearrange("p (n k) -> p n k", n=NSH),
                    S[:].rearrange("p (n k) -> p n k", n=NSH),
                    g_[:].rearrange("p (n o) -> p n o", o=1)
                        .broadcast_to([B, NSH, DC]),
                )
                return OUT

            rep_mm_prev = [None]
            mm_last_ref = [None]

            def replicate(OUTb, it):
                """OUTb [B, NK] bf16 -> OUTr [128, NK] bf16 (row b -> 4b..4b+3)."""
                pr = ps_rep.tile([128, NK], f32, tag="rep")
                mm = nc.tensor.matmul(pr[:], bd4t_t[:], OUTb[:],
                                      start=True, stop=True)
                rep_mm_prev[0] = mm
                cp = nc.vector.tensor_copy(OUTr[:], pr[:])
                return mm, cp

            # ---------------- iter 1 (uniform routing: E=1) ----------------
            # fold c 32->1 over IHC on DVE, pinned after the last copy so the
            # scheduler cannot interleave it into the copy stream
            U1f = U1[:].rearrange("p (c nk) -> p c nk", c=16)
            U2f = U2[:].rearrange("p (c nk) -> p c nk", c=8)
            IHf = IH[:].rearrange("p c nk -> p c nk")
            for j in range(4):
                nc.vector.tensor_add(
                    U1f[:, 4 * j:4 * j + 4, :],
                    IHf[:, 4 * j:4 * j + 4, :],
                    IHf[:, CH // 2 + 4 * j:CH // 2 + 4 * j + 4, :],
                )
                if j == 1:
                    # chunks 0+1 ready at position 15: hoists into the loop
                    nc.vector.tensor_add(U2f[:, 0:4, :], U1f[:, 0:4, :],
                                         U1f[:, 4:8, :])
            nc.vector.tensor_add(U2f[:, 4:8, :], U1f[:, 8:12, :],
                                 U1f[:, 12:16, :])
            nc.vector.tensor_add(U1f[:, 0:4, :], U2f[:, 0:4, :],
                                 U2f[:, 4:8, :])
            nc.vector.tensor_add(U2f[:, 0:2, :], U1f[:, 0:2, :],
                                 U1f[:, 2:4, :])
            nc.vector.tensor_add(
                SCR[:, 0:NK].rearrange("p (o nk) -> p o nk", o=1),
                U2f[:, 0:1, :], U2f[:, 1:2, :])
            pS1 = ps_s1.tile([B, NK], f32)
            nc.tensor.matmul(pS1[:], bd4_t[:], SCR[:, 0:NK],
                             start=True, stop=True)
            S1 = work.tile([B, NK], f32, tag="S")
            nc.vector.scalar_tensor_tensor(
                out=S1[:], in0=pS1[:], scalar=1.0 / IN, in1=brep_t[:],
                op0=MULT, op1=ADD,
            )
            OUT1 = squash(S1, 1)
            rep_mm, rep_cp = replicate(OUT1, 1)

            TMPk = TMP[:].rearrange("p (c n k) -> p c n k", c=CH, n=NSH)
            TMPc = TMP[:].rearrange("p (n k c) -> p n k c", n=NSH, k=DC)
            U1k = U1[:].rearrange("p (c n k) -> p c n k", c=CH, n=NSH)
            U2k = U2[:].rearrange("p (c n k) -> p c n k", c=CH, n=NSH)
            U1c = U1[:].rearrange("p (n k c) -> p n k c", n=NSH, k=DC)
            U2c = U2[:].rearrange("p (n k c) -> p n k c", n=NSH, k=DC)

            for it in (2, 3):
                # ---- a-step: TMP = IH * OUTr ; A = tree-fold k ----
                nc.vector.tensor_mul(
                    TMP[:].rearrange("p (c nk) -> p c nk", c=CH),
                    IH[:].rearrange("p c nk -> p c nk"),
                    OUTr[:].rearrange("p (o nk) -> p o nk", o=1)
                          .broadcast_to([128, CH, NK]),
                )
                nc.vector.tensor_add(U1k[:, :, :, 0:32], TMPk[:, :, :, 0:32],
                                     TMPk[:, :, :, 32:64])
                nc.vector.tensor_add(U2k[:, :, :, 0:16], U1k[:, :, :, 0:16],
                                     U1k[:, :, :, 16:32])
                nc.vector.tensor_add(U1k[:, :, :, 0:8], U2k[:, :, :, 0:8],
                                     U2k[:, :, :, 8:16])
                nc.vector.tensor_add(U2k[:, :, :, 0:4], U1k[:, :, :, 0:4],
                                     U1k[:, :, :, 4:8])
                nc.vector.tensor_add(U1k[:, :, :, 0:2], U2k[:, :, :, 0:2],
                                     U2k[:, :, :, 2:4])
                At = A2 if it == 2 else A3
                nc.vector.tensor_add(
                    At[:].rearrange("p (c n o) -> p c n o", c=CH, o=1),
                    U1k[:, :, :, 0:1], U1k[:, :, :, 1:2],
                )
                if it == 2:
                    BL = A2
                else:
                    BL = A3
                    nc.vector.tensor_add(A3[:], A3[:], A2[:])
                # ---- E = exp(BL), transposed write to [p, (n, c)] ----
                absorb("s", At[:2, :2])         # ACT clock <- tree (DVE)
                nc.scalar.activation(
                    E[:].rearrange("p (n c) -> p c n", n=NSH),
                    BL[:].rearrange("p (c n) -> p c n", c=CH),
                    AF.Exp,
                )
                # ---- Zp = sum_c E -> SCR[384:390] ----
                absorb("v", E[:2, :2])          # DVE clock <- E (ACT)
                with nc.allow_low_precision(reason="Z normalizer, positive sum"):
                    nc.vector.tensor_reduce(
                        SCR[:, NK:NK + NSH],
                        E[:].rearrange("p (n c) -> p n c", n=NSH),
                        axis=AX, op=ADD,
                    )
                # pz = BD4^T Zp early on the idle PE (disjoint region of the
                # pS bank); its reciprocal runs during mul2, off the tail
                pS = ps_s.tile([B, NK + NSH], f32, tag="pS")
                nc.tensor.matmul(pS[:, NK:NK + NSH], bd4_t[:],
                                 SCR[:, NK:NK + NSH], start=True, stop=True,
                                 skip_group_check=True)
                # ---- s-step: TMP2 = IHC * E ; P2 = tree-fold c ----
                if it == 2:
                    gsc2 = small.tile([2, 2], bf, tag="gihc")
                    ga2 = nc.vector.tensor_copy(gsc2[:], IHC[:2, :2, ch_order[2]])
                    add_dep_helper(ga2.ins, gp_ihc_last.ins, sync=True,
                                   reason="DVE clock <- gp IHC copies")
                m2v = nc.vector.tensor_mul(
                    TMPc,
                    IHC[:, :, 0:CH]
                       .rearrange("p (n k) c -> p n k c", n=NSH),
                    E[:].rearrange("p (n o c) -> p n o c", n=NSH, o=1)
                       .broadcast_to([128, NSH, DC, CH]),
                )
                if it == 2:
                    add_dep_helper(m2v.ins, ga2.ins, sync=False,
                                   reason="gp absorb before mul2")
                Rz = small.tile([B, NSH], f32, tag="Rz")
                nc.vector.reciprocal(Rz[:], pS[:, NK:NK + NSH])
                nc.vector.tensor_add(U1c[:, :, :, 0:16], TMPc[:, :, :, 0:16],
                                     TMPc[:, :, :, 16:32])
                nc.vector.tensor_add(U2c[:, :, :, 0:8], U1c[:, :, :, 0:8],
                                     U1c[:, :, :, 8:16])
                nc.vector.tensor_add(U1c[:, :, :, 0:4], U2c[:, :, :, 0:4],
                                     U2c[:, :, :, 4:8])
                nc.vector.tensor_add(U2c[:, :, :, 0:2], U1c[:, :, :, 0:2],
                                     U1c[:, :, :, 2:4])
                nc.vector.tensor_add(
                    SCR[:, 0:NK].rearrange("p (n k o) -> p n k o", n=NSH, o=1),
                    U2c[:, :, :, 0:1], U2c[:, :, :, 1:2])
                # ---- pS = BD4^T P2 ----
                mm_last = nc.tensor.matmul(pS[:, 0:NK], bd4_t[:],
                                           SCR[:, 0:NK], start=True, stop=True,
                                           skip_group_check=True)
                mm_last_ref[0] = mm_last
                # ---- S = pS/Z + brep ----
                absorb("v", pS[:2, :2])         # DVE clock <- pS (PE)
                Su = work.tile([B, NK], f32, tag="Su2")
                nc.vector.tensor_mul(
                    Su[:].rearrange("p (n k) -> p n k", n=NSH),
                    pS[:, 0:NK].rearrange("p (n k) -> p n k", n=NSH),
                    Rz[:].rearrange("p (n o) -> p n o", o=1)
                        .broadcast_to([B, NSH, DC]),
                )
                S = work.tile([B, NK], f32, tag="S")
                nc.vector.tensor_add(S[:], Su[:], brep_t[:])
                OUT = squash(S, it)
                if it < 3:
                    rep_mm, rep_cp = replicate(OUT, it)
                else:
                    # absorb stream/cst DMA queue sems into SYNC first so the
                    # out-DMA's queue-reuse wait dedups to a single sem
                    for fin in (c_dma, *s_dmas):
                        fnop = nc.sync.nop()
                        add_dep_helper(fnop.ins, fin.ins, sync=True,
                                       reason="absorb DMA sem for queue reuse")
                    o_dma = nc.sync.dma_start(out=out_d[:], in_=OUT[:])
                    f_scr = small.tile([2, 4], f32, tag="fin")
                    f_act = nc.scalar.copy(f_scr[:, 0:2], OUT[:2, :2])
                    f_dve = nc.vector.tensor_copy(f_scr[:, 2:4], OUT[:2, :2])
                    for fin in (mm_last, f_act, f_dve, o_dma):
                        fnop = nc.sync.nop()
                        add_dep_helper(fnop.ins, fin.ins, sync=True,
                                       reason="absorb final sem for tail drain")

    return nc


def _pack_inputs(inputs, W, B_param):
    """Host-side shard + relayout. Returns list of 8 in_maps."""
    import ml_dtypes
    bf16 = ml_dtypes.bfloat16
    inputs = np.ascontiguousarray(inputs, dtype=np.float32)
    W = np.ascontiguousarray(W, dtype=np.float32)
    B_param = np.ascontiguousarray(B_param, dtype=np.float32)

    Wp = np.zeros((CH, NCP, DC, DIN), dtype=np.float32)
    Wp[:, :NC] = W
    Bp = np.zeros((NCP, DC), dtype=np.float32)
    Bp[:NC] = B_param

    # xt[(c,dc), dd, (b,rr)] = x[b, 4c+rr, 128dc+dd]
    x4 = inputs.reshape(B, CH, 4, 2, 128)           # b, c, rr, dc, dd
    xt = x4.transpose(1, 3, 4, 0, 2).reshape(CH * 2, 128, 128)
    bd4 = np.zeros((128, B), dtype=np.float32)
    bd4[np.arange(128), np.arange(128) // 4] = 1.0
    bd4t = bd4.T

    in_maps = []
    for core in range(NCORES):
        sl = slice(core * NSH, (core + 1) * NSH)
        Wc = Wp[:, sl]                               # c, n, k, d
        w5 = Wc.reshape(CH, NSH, DC, 2, 128)         # c n k dc dd
        wtc = w5.transpose(0, 3, 4, 1, 2).reshape(CH * 2, 128, NK)
        cstc = np.zeros((128, 544), dtype=np.float32)
        cstc[:, 0:B] = bd4
        cstc[0:B, B:B + 128] = bd4t
        cstc[0:B, B + 128:B + 128 + NK] = np.broadcast_to(
            Bp[sl].reshape(1, NK), (B, NK))
        xwc = np.concatenate([xt, wtc], axis=2)      # [64, 128, 512]
        # reorder chunks to the kernel's interleaved channel order
        ch_order = []
        for t in range(CH // 2):
            ch_order += [t, t + CH // 2]
        perm = np.zeros(CH * 2, dtype=np.int64)
        for p_, c_ in enumerate(ch_order):
            perm[2 * p_] = 2 * c_
            perm[2 * p_ + 1] = 2 * c_ + 1
        xwc = xwc[perm]
        # partition-major: [d, cd, 512] for 32KB-contiguous DMA runs
        xwc = np.ascontiguousarray(xwc.transpose(1, 0, 2)).astype(bf16)
        in_maps.append(dict(xw=xwc, cst=cstc.astype(bf16)))
    return in_maps


def _run(inputs, W, B_param, trace=False):
    from concourse.bass_utils import run_bass_kernel_spmd

    if "nc" not in _cache:
        _cache["nc"] = _build_nc()
    nc = _cache["nc"]
    in_maps = _pack_inputs(inputs, W, B_param)
    res = run_bass_kernel_spmd(nc, in_maps, core_ids=list(range(NCORES)),
                               trace=trace)
    outs = [r["out"].reshape(B, NSH, DC) for r in res.results]
    full = np.concatenate(outs, axis=1)[:, :NC, :]
    return np.ascontiguousarray(full.astype(np.float32)), res


def kernel(inputs, W, B_param):
    out, _ = _run(inputs, W, B_param, trace=False)
    return out
